# revision 77
# baseline (speedup 1.0000x reference)
"""Bahdanau attention Trainium2 kernel.

Math: out = softmax_k(mask(score)) @ values with
  score[b,q,k] = sum_h wv[h] * tanh(Q[b,q,h] + K[b,k,h]),
  Q = queries @ wq, K = keys @ wk.

tanh(x) is approximated by a mixed polynomial + free-frequency sine
basis
  tanh(x) ~= c1*x + c3*x^3 + sum_m alpha_m sin(omega_m x)
(coefficients and omegas jointly optimized per fit range offline,
hardcoded below). Each sine term factorizes through
sin(omega(q+k)) = sin(wq)cos(wk) + cos(wq)sin(wk) into dense
[Tq,H]x[H,Tk] matmuls on the PE array; the polynomial part expands
binomially: pure-q terms are constant along the softmax axis and
cancel, the pure-k rank-1 part (c1*k + c3*k^3 contracted with wv) is
precomputed on the host and folded into the additive-mask rows, and
only the cross terms 3c3*(q^2 k + q k^2) run on device as fp16 PE
matmul channels. This leaves just TWO Sin evaluations on the
activation engine, which is the critical resource.

The scalar engine's Sin is only valid on [-pi, pi], so arguments are
range-reduced in 16-bit fixed point using the HW's round-to-nearest
fp32->int32 conversion:
  n = round(x * omega/(2pi) * 65536)     (one tensor_scalar, int32 out)
ACT then reads the LOW int16 half of each int32 via a bitcast +
stride-2 AP; the SIGNED int16 view puts the phase in [-pi, pi)
directly (sin arg = v16 * 2pi/65536, no bias).

Scores are accumulated TRANSPOSED (scoresT[k, q], k on partitions, two
128-row chunks per batch) by swapping matmul lhsT/rhs. This removes the
tail transpose entirely and lets the masked softmax fold completely into
the Exp activation: exp(score*scale + bias) with per-partition
scale/bias tiles carrying the valid_len mask (masked k rows get
bias=-1e6 -> exp underflows to 0; rows with valid_len==0 get scale=0,
bias=0 -> uniform attention, matching the reference). e is written in
fp16 so the attn@values and row-sum matmuls run at full PE rate; the
1/sum normalization is applied per-partition on the final PSUM->SBUF
copy.

All PE inputs (wq/wk, qT/kT, trig factors, e, values) are fp16:
1 cycle/row on the PE vs 4 for fp32, and half the DMA bytes.

Sharding: data-parallel over batch, 2 batches per core on 8 cores.
"""

import math
import sys

import numpy as np

sys.path.insert(0, "/opt/trn_rl_repo")

B, TQ, TK, DIN, H, DV = 16, 128, 256, 64, 256, 256
NCORES = 8
NB = B // NCORES
HB = 2  # h blocks of 128 partitions
NEG = -1000000.0
PI = math.pi
FX = 65536  # fixed-point phase resolution

# (fit range R, [alpha_m], [omega_m]) — offline weighted least squares with
# jointly optimized frequencies (Nelder-Mead over log omega).
FITS = [
    # R=4.25 mixed basis {x, x^3, sin w1 x, sin w2 x}: the polynomial part
    # factorizes into cheap PE channels (binomial expansion; pure-q terms
    # cancel in the softmax), leaving only TWO Sin evaluations on ACT.
    (4.25, [0.02924106, 0.20011121], [2.98939508, 1.61959803],
     0.57994674, -0.02086779),
    # R=4.25 M=3, weights tuned for end-to-end error on randn inputs
    (4.25, [1.14215814, 0.16947486, 0.02756034], [0.54450034, 1.69464501, 3.01720302]),
    # R=4.35 M=4 fit_max=2.65e-03
    (4.35, [1.15424910, 0.18579149, 0.03183397, 0.00486786], [0.51479938, 1.59132282, 2.77293209, 4.09736273]),
    # R=5.0 M=5 fit_max=1.15e-03
    (5.0, [1.17638592, 0.21722942, 0.04686229, 0.00903701, 0.00168118], [0.45910491, 1.40743013, 2.42317614, 3.51883170, 4.71208021]),
    # R=5.75 M=5 fit_max=2.69e-03
    (5.75, [1.19166771, 0.24153365, 0.06066643, 0.01419093, 0.00340621], [0.41816285, 1.27578909, 2.18259325, 3.14991184, 4.18200973]),
    # R=6.5 M=6 fit_max=2.18e-03
    (6.5, [1.22246911, 0.29583495, 0.09660808, 0.02984660, 0.00854484, 0.00257873], [0.33487488, 1.02337718, 1.75741117, 2.54559399, 3.38983334, 4.27904363]),
    # R=7.5 M=7 fit_max=2.83e-03
    (7.5, [1.23017784, 0.32207557, 0.13281313, 0.05605996, 0.02115424, 0.00734471, 0.00271091], [0.27799558, 0.82769422, 1.38567976, 1.99585889, 2.67159272, 3.40357483, 4.16577676]),
    # R=9.0 M=8 fit_max=3.42e-03
    (9.0, [1.23220768, 0.32660174, -0.09537761, 0.19814386, 0.04758848, 0.01766320, 0.00668698, 0.00282454], [0.29504675, 0.89849354, 1.31818698, 1.42932216, 2.09287727, 2.73674004, 3.39807058, 4.05802853]),
    # R=11.0 M=10 fit_max=2.34e-03
    (11.0, [1.25009981, 0.36156082, 0.16668645, 0.08288622, 0.04121021, 0.02000684, 0.00953954, 0.00468461, 0.00244755, -0.00000540], [0.21481795, 0.64601870, 1.08287839, 1.53102994, 1.99552922, 2.47865898, 2.98018571, 3.49646094, 4.00027643, 27.89010198]),
    # R=13.5 M=12 fit_max=1.65e-03
    (13.5, [1.25071458, 0.35628586, 0.11460491, 0.10187407, 0.14817974, -0.36003900, 0.32427418, 0.02462988, 0.01227283, 0.00633834, 0.00344839, 0.00201855], [0.19982948, 0.59368416, 0.91929799, 1.13521056, 1.56105869, 1.71511767, 1.75607839, 2.27997771, 2.72042538, 3.16077613, 3.60347782, 4.03539051]),
    # R=16.5 M=14 fit_max=2.77e-03
    (16.5, [1.25849188, 0.38272733, 0.19240301, 0.10549262, 0.02066639, 0.04720551, 0.04224601, -0.03774592, 0.04760619, 0.01225844, 0.00687190, 0.00399597, 0.01283338, -0.03588798, 0.02641497, 0.00090983, 0.00029855], [0.17064909, 0.51252204, 0.85576113, 1.19857872, 1.46425403, 1.59143819, 1.94973110, 2.13344383, 2.20286683, 2.61053434, 2.98029452, 3.35535440, 3.82535729, 3.89323910, 3.92054336, 4.37490434, 6.06447818]),
    # R=20.0 M=24 fit_max=8.08e-03
    (20.0, [1.26310001, 0.39699981, 0.50851750, -0.19663028, -0.15967006, 0.16325805, 0.07906020, 0.04867188, 0.03041830, 0.01901565, 0.01861664, -0.00714429, 0.00795889, 0.00512261, 0.00317850, 0.00126551, 0.00070946, 0.00065840, 0.00017538, -0.00000124, -0.00006095, -0.00011474, 0.00012987, -0.00002543], [0.14670123, 0.44208486, 0.76888929, 0.77629792, 0.84403806, 0.98925559, 1.30465383, 1.60553746, 1.90541734, 2.20846370, 2.53020022, 2.56316326, 2.81407758, 3.11334895, 3.41541978, 4.41970952, 4.63745933, 5.05356013, 6.39364912, 6.87284072, 7.81029508, 10.65968755, 10.76481947, 18.82746599]),
]

PQK = 2 * H + NB * TQ + NB * TK  # wq | qT | wk | kT   (fp16, 64 partitions)


def build_program(alpha, omega, exp_shift=0.0, c1=0.0, c3=0.0):
    """Build the per-core Bass program."""
    import concourse.bacc as bacc
    import concourse.bass as bass
    import concourse.mybir as mybir
    import concourse.tile as tile

    f32 = mybir.dt.float32
    f16 = mybir.dt.float16
    i32 = mybir.dt.int32
    i16 = mybir.dt.int16
    AF = mybir.ActivationFunctionType
    ALU = mybir.AluOpType

    M = len(alpha)
    mixed = c3 != 0.0
    CC = HB * M + (HB if mixed else 0)  # sine folds | 3c3*wv
    PQKX = PQK

    nc = bacc.Bacc("TRN2", target_bir_lowering=False, debug=False)

    QHALF = H + NB * TQ  # wq | qT
    pack64a_d = nc.dram_tensor("pack64a", [DIN, QHALF], f16, kind="ExternalInput").ap()
    pack64b_d = nc.dram_tensor("pack64b", [DIN, PQKX - QHALF], f16, kind="ExternalInput").ap()
    packc_d = nc.dram_tensor("packc", [128, CC], f32, kind="ExternalInput").ap()
    cmask_d = nc.dram_tensor("cmask", [1, NB * 2 * 128], f16, kind="ExternalInput").ap()
    packv_d = nc.dram_tensor("packv", [128, NB * 2 * DV], f16, kind="ExternalInput").ap()
    out_d = nc.dram_tensor("out", [NB, TQ, DV], f32, kind="ExternalOutput").ap()

    with tile.TileContext(nc) as tc:
        with (
            tc.tile_pool(name="singles", bufs=1) as singles,
            tc.tile_pool(name="trig", bufs=3) as trig,
            tc.tile_pool(name="soft", bufs=2) as soft,
            tc.tile_pool(name="pproj", bufs=1, space="PSUM") as pproj,
            tc.tile_pool(name="pscore", bufs=1, space="PSUM") as pscore,
            tc.tile_pool(name="ptail", bufs=1, space="PSUM") as ptail,
        ):
            # ---- constants / inputs to SBUF ----
            warm_t = singles.tile([128, 1], f32)
            nc.vector.memset(warm_t, 0.0)
            ones16 = singles.tile([128, 1], f16)
            nc.vector.memset(ones16, 1.0)
            zeros16 = singles.tile([128, 1], f16)
            nc.vector.memset(zeros16, 0.0)
            onesrow = singles.tile([1, TQ], f16)
            nc.vector.memset(onesrow, 1.0)
            bias_exp = singles.tile([128, 1], f32)
            nc.vector.memset(bias_exp, -float(exp_shift))

            pk64 = singles.tile([DIN, PQKX], f16)
            # q-half first: its (smaller) transfer completes sooner, so the
            # q projection and the scalar-engine copy chain start earlier
            nc.sync.dma_start(out=pk64[:, 0:QHALF], in_=pack64a_d)
            nc.sync.dma_start(out=pk64[:, QHALF:], in_=pack64b_d)
            pc = singles.tile([128, CC], f32)
            nc.sync.dma_start(out=pc, in_=packc_d)
            cmask = singles.tile([1, NB * 2 * 128], f16)
            nc.sync.dma_start(out=cmask, in_=cmask_d)
            pv = singles.tile([128, NB, 2, DV], f16)
            nc.sync.dma_start(
                out=pv,
                in_=bass.AP(tensor=packv_d.tensor, offset=0, ap=[[NB * 2 * DV, 128], [1, NB * 2 * DV]]),
            )

            wq_sb = pk64[:, 0:H]
            qTs = pk64[:, H : H + NB * TQ].rearrange("p (b x) -> p b x", b=NB)
            wk_sb = pk64[:, H + NB * TQ : 2 * H + NB * TQ]
            kTs = pk64[:, 2 * H + NB * TQ : PQK].rearrange(
                "p (b x) -> p b x", b=NB
            )
            fold_sb = pc[:, 0 : HB * M].rearrange("p (hb m) -> p hb m", hb=HB)

            # dummy Sin before the scalar-engine copies: pins the first
            # (startup-hidden) act-table load to the trig table so the Copy
            # activations below don't cause an extra mid-program load
            dummy_sin = singles.tile([128, 1], f16)
            nc.scalar.activation(
                out=dummy_sin, in_=warm_t, func=AF.Sin, bias=0.0, scale=1.0
            )

            # ---- projections (PE, fp16 in / fp32 psum): [h, hb, b, qi/ki] ----
            # tiny warm-up matmul first (into the qT region, re-zeroed by the
            # real projection's start): begins the PE p-state ramp early
            qT_ps = pproj.tile([128, HB, NB, TQ], f32)
            nc.tensor.matmul(
                qT_ps[0:1, 0, 0, 0:1], lhsT=warm_t, rhs=warm_t,
                start=True, stop=True, skip_group_check=True,
            )
            for hb in range(HB):
                nc.tensor.matmul(
                    qT_ps[:, hb, :, :].rearrange("p b x -> p (b x)"),
                    lhsT=wq_sb[:, hb * 128 : (hb + 1) * 128],
                    rhs=qTs.rearrange("p b x -> p (b x)"),
                    start=(hb == 0),
                    stop=(hb == HB - 1),
                )
            # kT_ps spans two 2KB psum zero regions (one per hb slice);
            # start/stop must bracket each region's writes separately.
            kT_ps = pproj.tile([128, HB, NB, TK], f32)
            for hb in range(HB):
                for b in range(NB):
                    nc.tensor.matmul(
                        kT_ps[:, hb, b, :],
                        lhsT=wk_sb[:, hb * 128 : (hb + 1) * 128],
                        rhs=kTs[:, b, :],
                        start=(b == 0),
                        stop=(b == NB - 1),
                    )

            # both PSUM->SBUF copies run on the (otherwise idle) scalar
            # engine so the DVE goes straight to the m=0 phase converts
            qTp = singles.tile([128, HB, NB, TQ], f32)
            nc.scalar.copy(out=qTp, in_=qT_ps)
            kTp = singles.tile([128, HB, NB, TK], f32)
            nc.scalar.copy(out=kTp, in_=kT_ps)

            # ---- transposed score accumulation over m sine terms ----
            # sT[b][k', kc, q] : k = kc*128 + k' on partitions. One 2KB psum
            # bank (= one zero region) per batch; the kc slices interleave
            # inside a single accumulation group bracketed by the batch's
            # first (start) and last (stop) matmul, like the baseline's
            # kT_ps hb regions.
            sT = [
                pscore.tile([128, 2, TQ], f32, tag=f"sT{b}", name=f"sT{b}")
                for b in range(NB)
            ]
            # per batch: 2 mask adds [+ 8 cubic channels] + sines
            n_mm = M * HB * 2 * 2 + 2 + (8 if mixed else 0)
            mm_i = [0] * NB
            sin_scale = 2.0 * PI / FX

            # additive softmax mask seeded into the score accumulators by
            # tiny 1-partition matmuls (c[k] x ones[q]) while the PE is
            # otherwise idle; runs right after the input DMA lands
            for b in range(NB):
                for kc in range(2):
                    r0 = (b * 2 + kc) * 128
                    nc.tensor.matmul(
                        sT[b][:, kc, :],
                        lhsT=cmask[0:1, r0 : r0 + 128],
                        rhs=onesrow,
                        start=(mm_i[b] == 0),
                        stop=False,
                    )
                    mm_i[b] += 1

            # ---- polynomial part of the mixed tanh basis ----
            # c1*x + c3*x^3 with x = q+k expands binomially: pure-q terms
            # cancel in the softmax, the pure-k rank-1 (c1*k + c3*k^3 through
            # wv) is folded into the cmask rows on the HOST, and only the
            # cross channels 3c3*(q^2 k + q k^2) run here as f16 PE matmuls.
            if mixed:
                f3a = pc[:, HB * M : HB * M + HB]  # 3*c3*wv per hb
                q2 = singles.tile([128, HB, NB, TQ], f32)
                k2 = singles.tile([128, HB, NB, TK], f32)
                kp16 = singles.tile([128, HB, NB, TK], f16)
                k2_16 = singles.tile([128, HB, NB, TK], f16)
                fq2 = singles.tile([128, HB, NB, TQ], f16)
                fq1 = singles.tile([128, HB, NB, TQ], f16)

            def emit_poly_powers():
                # q square on gpsimd (ready before the m0 AC folds need
                # Pool); k square on the DVE after the m0 converts
                nc.gpsimd.tensor_tensor(out=q2, in0=qTp, in1=qTp, op=ALU.mult)

            def emit_poly_folds():
                for hb in range(HB):
                    nc.gpsimd.tensor_scalar(
                        out=fq2[:, hb], in0=q2[:, hb],
                        scalar1=f3a[:, hb : hb + 1], scalar2=None, op0=ALU.mult,
                    )
                    nc.gpsimd.tensor_scalar(
                        out=fq1[:, hb], in0=qTp[:, hb],
                        scalar1=f3a[:, hb : hb + 1], scalar2=None, op0=ALU.mult,
                    )
                nc.vector.tensor_tensor(out=k2, in0=kTp, in1=kTp, op=ALU.mult)
                nc.vector.tensor_copy(out=kp16, in_=kTp)
                nc.vector.tensor_copy(out=k2_16, in_=k2)

            def emit_poly_matmuls():
                for b in range(NB):
                    for kc in range(2):
                        ks = slice(kc * 128, (kc + 1) * 128)
                        for hb in range(HB):
                            nc.tensor.matmul(
                                sT[b][:, kc, :],
                                lhsT=kp16[:, hb, b, ks],
                                rhs=fq2[:, hb, b, :],
                                start=False, stop=False,
                            )
                            mm_i[b] += 1
                            nc.tensor.matmul(
                                sT[b][:, kc, :],
                                lhsT=k2_16[:, hb, b, ks],
                                rhs=fq1[:, hb, b, :],
                                start=False, stop=False,
                            )
                            mm_i[b] += 1

            def emit_phase(m):
                """DVE int phase converts + ACT Sin for sine term m.

                Phase tiles hold [hb, sin|cos, b, x] so one ACT Sin per side
                sweeps both quadratures. ACT reads the LOW int16 half of each
                int32 via a bitcast + stride-2 AP; the SIGNED int16 view puts
                the phase in [-pi, pi) directly."""
                w16 = float(np.float32(omega[m] / (2.0 * PI) * FX))
                nq = trig.tile([128, HB, 2, NB, TQ], i32, tag="nq", name="nq")
                nc.vector.tensor_scalar(
                    out=nq[:, :, 0], in0=qTp, scalar1=w16, scalar2=None, op0=ALU.mult
                )
                nc.vector.tensor_scalar(
                    out=nq[:, :, 1], in0=qTp, scalar1=w16, scalar2=float(FX // 4),
                    op0=ALU.mult, op1=ALU.add,
                )
                nk = trig.tile([128, HB, 2, NB, TK], i32, tag="nk", name="nk")
                nc.vector.tensor_scalar(
                    out=nk[:, :, 0], in0=kTp, scalar1=w16, scalar2=None, op0=ALU.mult
                )
                nc.vector.tensor_scalar(
                    out=nk[:, :, 1], in0=kTp, scalar1=w16, scalar2=float(FX // 4),
                    op0=ALU.mult, op1=ALU.add,
                )
                # t[:,hb,0] = sin(w x), t[:,hb,1] = cos(w x)
                tq = trig.tile([128, HB, 2, NB, TQ], f16, tag="tq", name="tq")
                nc.scalar.activation(
                    out=tq, in_=nq.bitcast(i16)[:, :, :, :, 0::2], func=AF.Sin,
                    bias=0.0, scale=sin_scale,
                )
                tk = trig.tile([128, HB, 2, NB, TK], f16, tag="tk", name="tk")
                nc.scalar.activation(
                    out=tk, in_=nk.bitcast(i16)[:, :, :, :, 0::2], func=AF.Sin,
                    bias=0.0, scale=sin_scale,
                )
                # zero-contribution keep-warm matmuls, one per fresh trig
                # tile: they space out through the mainloop and stop the PE
                # p-state ramp from resetting during its long idle, so the
                # final matmul burst runs at full clock
                for rhs in (tq[:, 0, 0, 0, 0:1], tk[:, 0, 0, 0, 0:1]):
                    nc.tensor.matmul(
                        sT[0][0:1, 0, 0:1], lhsT=zeros16, rhs=rhs,
                        start=False, stop=False, skip_group_check=True,
                    )
                return tq, tk

            def emit_reduce(m, tq, tk):
                """Fold alpha_m*wv into the q side, then accumulate the
                transposed scores: sT[b][kc] += tkc.T @ (fold*sin q)
                + tks.T @ (fold*cos q)."""
                AC = trig.tile([128, HB, 2, NB, TQ], f16, tag="AC", name="AC")
                for hb in range(HB):
                    nc.gpsimd.tensor_scalar(
                        out=AC[:, hb], in0=tq[:, hb],
                        scalar1=fold_sb[:, hb, m : m + 1], scalar2=None, op0=ALU.mult,
                    )
                for b in range(NB):
                    for kc in range(2):
                        for hb in range(HB):
                            ks = slice(kc * 128, (kc + 1) * 128)
                            nc.tensor.matmul(
                                sT[b][:, kc, :],
                                lhsT=tk[:, hb, 1, b, ks],
                                rhs=AC[:, hb, 0, b, :],
                                start=(mm_i[b] == 0),
                                stop=(mm_i[b] == n_mm - 1),
                            )
                            mm_i[b] += 1
                            nc.tensor.matmul(
                                sT[b][:, kc, :],
                                lhsT=tk[:, hb, 0, b, ks],
                                rhs=AC[:, hb, 1, b, :],
                                start=(mm_i[b] == 0),
                                stop=(mm_i[b] == n_mm - 1),
                            )
                            mm_i[b] += 1

            # software-pipeline: folds+matmuls for term m are emitted after
            # phase m+1, so gpsimd's AC(m) doesn't stall the DVE->ACT chain.
            # The polynomial pieces slot in behind the m=0 emissions.
            pending = None
            for m in range(M):
                cur = emit_phase(m)
                if mixed and m == 0:
                    emit_poly_powers()
                if pending is not None:
                    emit_reduce(*pending)
                    if mixed and pending[0] == 0:
                        emit_poly_folds()
                        emit_poly_matmuls()
                pending = (m, *cur)
            emit_reduce(*pending)

            # ---- masked softmax + attn @ values, per batch ----
            # The mask lives entirely in the Exp activation's per-partition
            # scale/bias (k is the partition dim of sT): masked rows get
            # bias=-1e6 (exp -> exactly 0), valid_len==0 batches get
            # scale=0,bias=0 (uniform attention). exp_shift is folded into
            # bias host-side. Row sums over k via a ones-matmul; the 1/sum
            # is applied per-partition (q) on the PSUM->SBUF output copy.
            out_sb = soft.tile([128, NB, DV], f32, tag="out_sb", name="out_sb")
            e16 = soft.tile([128, NB, 2, TQ], f16, tag="e16", name="e16")
            for b in range(NB):
                nc.scalar.activation(
                    out=e16[:, b], in_=sT[b], func=AF.Exp,
                    bias=bias_exp, scale=1.0,
                )
            # per-batch accumulator tile: value columns + a sums column in
            # one psum bank, so each batch's group stops at its OWN last
            # matmul and its reciprocal/normalize isn't gated on the other
            # batch. Order per batch: value kc0 (start) ... sums kc1 (stop).
            tails = [
                ptail.tile([128, DV + 1], f32, tag=f"tail{b}", name=f"tail{b}")
                for b in range(NB)
            ]
            for b in range(NB):
                for kc in range(2):
                    nc.tensor.matmul(
                        tails[b][:, 0:DV],
                        lhsT=e16[:, b, kc, :],
                        rhs=pv[:, b, kc, :],
                        start=(kc == 0),
                        stop=False,
                    )
                    nc.tensor.matmul(
                        tails[b][:, DV : DV + 1],
                        lhsT=e16[:, b, kc, :],
                        rhs=ones16,
                        start=False,
                        stop=(kc == 1),
                    )
            # b1's normalize runs on the scalar engine (Identity with a
            # per-partition scale) so it isn't queued behind b0's on DVE;
            # separate out tiles avoid a cross-engine whole-tile WAW stall.
            rr = soft.tile([128, NB], f32, tag="r", name="r")
            nc.vector.reciprocal(out=rr[:, 0:1], in_=tails[0][:, DV : DV + 1])
            nc.vector.tensor_scalar(
                out=out_sb[:, 0, :], in0=tails[0][:, 0:DV], scalar1=rr[:, 0:1],
                scalar2=None, op0=ALU.mult,
            )
            nc.sync.dma_start(out=out_d[0], in_=out_sb[:, 0, :])
            out_sb1 = soft.tile([128, DV], f32, tag="out_sb1", name="out_sb1")
            nc.vector.reciprocal(out=rr[:, 1:2], in_=tails[1][:, DV : DV + 1])
            nc.vector.tensor_scalar(
                out=out_sb1, in0=tails[1][:, 0:DV], scalar1=rr[:, 1:2],
                scalar2=None, op0=ALU.mult,
            )
            nc.sync.dma_start(out=out_d[1], in_=out_sb1)

    nc.compile()
    return nc


def prepare_in_maps(queries, keys, values, valid_lens, wq, wk, wv, alpha,
                    exp_shift, c1=0.0, c3=0.0):
    """Host-side sharding + layout transforms. Returns list of 8 input dicts."""
    M = len(alpha)
    mixed = c3 != 0.0
    queries = np.ascontiguousarray(queries, dtype=np.float32)
    keys = np.ascontiguousarray(keys, dtype=np.float32)
    values = np.ascontiguousarray(values, dtype=np.float32)
    wq16 = np.ascontiguousarray(wq, dtype=np.float16)
    wk16 = np.ascontiguousarray(wk, dtype=np.float16)
    wv = np.asarray(wv, dtype=np.float32)
    valid_lens = np.asarray(valid_lens)

    # fold[p, hb, m] = alpha_m * wv[hb*128 + p]
    fold = np.empty((128, HB, M), np.float32)
    for hb in range(HB):
        fold[:, hb, :] = np.asarray(alpha, np.float64)[None, :] * wv[
            hb * 128 : (hb + 1) * 128, None
        ]

    CNEG = -60000.0  # f16-representable; exp underflows to exactly 0
    karange = np.arange(128)
    in_maps = []
    for c in range(NCORES):
        bs = slice(c * NB, (c + 1) * NB)
        qT = queries[bs].transpose(2, 0, 1).reshape(DIN, NB, TQ).copy()
        kT = keys[bs].transpose(2, 0, 1).reshape(DIN, NB, TK).copy()
        # additive mask rows per (b, kchunk); valid_len==0 batches get
        # zeroed q/k (scores==0 exactly) + zero mask -> uniform attention
        cmask = np.zeros((NB, 2, 128), np.float16)
        for j, vl in enumerate(valid_lens[bs]):
            vl = int(vl)
            if vl <= 0:
                qT[:, j] = 0.0
                kT[:, j] = 0.0
            else:
                for kc in range(2):
                    cmask[j, kc] = np.where(
                        (kc * 128 + karange) < vl, 0.0, CNEG
                    ).astype(np.float16)
        p64 = [wq16, qT.reshape(DIN, NB * TQ).astype(np.float16), wk16,
               kT.reshape(DIN, NB * TK).astype(np.float16)]
        pcc = [fold.reshape(128, HB * M)]
        if mixed:
            f3a = np.empty((128, HB), np.float32)
            for hb in range(HB):
                f3a[:, hb] = 3.0 * c3 * wv[hb * 128 : (hb + 1) * 128]
            pcc += [f3a]
            # pure-k part of the polynomial, precontracted with wv on the
            # host: vk[k] = sum_h wv_h * (c1*kp + c3*kp^3); folded into the
            # additive cmask rows (zeroed-key batches get vk == 0)
            kp = kT.reshape(DIN, NB * TK).T.astype(np.float64) @ wk.astype(
                np.float64
            )
            vk = (c1 * kp + c3 * kp**3) @ wv.astype(np.float64)
            cmask += vk.reshape(NB, 2, 128).astype(np.float16)
        pack64a = np.concatenate(p64[:2], axis=1)
        pack64b = np.concatenate(p64[2:], axis=1)
        vals = values[bs].reshape(NB, 2, 128, DV).transpose(2, 0, 1, 3)
        packv = vals.reshape(128, NB * 2 * DV).astype(np.float16)
        packc = np.concatenate(pcc, axis=1).astype(np.float32)
        in_maps.append(
            {
                "pack64a": np.ascontiguousarray(pack64a),
                "pack64b": np.ascontiguousarray(pack64b),
                "packc": np.ascontiguousarray(packc),
                "cmask": np.ascontiguousarray(cmask.reshape(1, NB * 2 * 128)),
                "packv": np.ascontiguousarray(packv),
            }
        )
    return in_maps


def _pick_fit(queries, keys, wq, wk):
    q = queries.reshape(-1, DIN).astype(np.float32) @ wq.astype(np.float32)
    k = keys.reshape(-1, DIN).astype(np.float32) @ wk.astype(np.float32)
    qb = q.reshape(B, TQ, H)
    kb = k.reshape(B, TK, H)
    hi = (qb.max(1) + kb.max(1)).max()
    lo = (qb.min(1) + kb.min(1)).min()
    r_needed = max(abs(hi), abs(lo))
    for ent in FITS:
        if ent[0] >= r_needed + 0.05:
            break
    else:
        ent = FITS[-1]
    R, alpha, omega = ent[0], ent[1], ent[2]
    c1 = ent[3] if len(ent) > 3 else 0.0
    c3 = ent[4] if len(ent) > 4 else 0.0
    return R, alpha, omega, c1, c3


_prog_cache = {}


def kernel(queries, keys, values, valid_lens, wq, wk, wv):
    from concourse import bass_utils

    queries = np.asarray(queries)
    keys = np.asarray(keys)
    values = np.asarray(values)
    valid_lens = np.asarray(valid_lens)
    wq = np.asarray(wq)
    wk = np.asarray(wk)
    wv = np.asarray(wv)

    R, alpha, omega, c1, c3 = _pick_fit(queries, keys, wq, wk)
    # scores bounded by ~sum|wv| * max|approx tanh|; keep exp(score) within
    # fp16 range (e is stored as fp16)
    bound = float(np.abs(wv).sum()) * 1.01
    exp_shift = max(0.0, bound - 10.0)

    key = (R, len(alpha), round(exp_shift, 3))
    if key not in _prog_cache:
        _prog_cache[key] = build_program(alpha, omega, exp_shift, c1, c3)
    nc = _prog_cache[key]

    in_maps = prepare_in_maps(
        queries, keys, values, valid_lens, wq, wk, wv, alpha, exp_shift, c1, c3
    )
    res = bass_utils.run_bass_kernel_spmd(nc, in_maps, core_ids=list(range(NCORES)))
    out = np.concatenate([r["out"] for r in res.results], axis=0)
    return out.astype(np.float32)


if __name__ == "__main__":
    rng = np.random.default_rng(0)
    inputs = {
        "queries": rng.standard_normal((B, TQ, DIN), dtype=np.float32),
        "keys": rng.standard_normal((B, TK, DIN), dtype=np.float32),
        "values": rng.standard_normal((B, TK, DV), dtype=np.float32),
        "valid_lens": rng.integers(0, TK, size=(B,)).astype(np.int32),
        "wq": (rng.standard_normal((DIN, H), dtype=np.float32) * 0.05),
        "wk": (rng.standard_normal((DIN, H), dtype=np.float32) * 0.05),
        "wv": (rng.standard_normal((H,), dtype=np.float32) * 0.05),
    }
    out = kernel(**inputs)
    print("out", out.shape, out.dtype)


# revision 79
# speedup vs baseline: 1.0218x; 1.0218x over previous
"""Bahdanau attention Trainium2 kernel.

Math: out = softmax_k(mask(score)) @ values with
  score[b,q,k] = sum_h wv[h] * tanh(Q[b,q,h] + K[b,k,h]),
  Q = queries @ wq, K = keys @ wk.

tanh(x) is approximated by a mixed polynomial + free-frequency sine
basis
  tanh(x) ~= c1*x + c3*x^3 + sum_m alpha_m sin(omega_m x)
(coefficients and omegas jointly optimized per fit range offline,
hardcoded below). Each sine term factorizes through
sin(omega(q+k)) = sin(wq)cos(wk) + cos(wq)sin(wk) into dense
[Tq,H]x[H,Tk] matmuls on the PE array; the polynomial part expands
binomially: pure-q terms are constant along the softmax axis and
cancel, the pure-k rank-1 part (c1*k + c3*k^3 contracted with wv) is
precomputed on the host and folded into the additive-mask rows, and
only the cross terms 3c3*(q^2 k + q k^2) run on device as fp16 PE
matmul channels. This leaves just TWO Sin evaluations on the
activation engine, which is the critical resource.

The scalar engine's Sin is only valid on [-pi, pi], so arguments are
range-reduced in 16-bit fixed point using the HW's round-to-nearest
fp32->int32 conversion:
  n = round(x * omega/(2pi) * 65536)     (one tensor_scalar, int32 out)
ACT then reads the LOW int16 half of each int32 via a bitcast +
stride-2 AP; the SIGNED int16 view puts the phase in [-pi, pi)
directly (sin arg = v16 * 2pi/65536, no bias).

Scores are accumulated TRANSPOSED (scoresT[k, q], k on partitions, two
128-row chunks per batch) by swapping matmul lhsT/rhs. This removes the
tail transpose entirely and lets the masked softmax fold completely into
the Exp activation: exp(score*scale + bias) with per-partition
scale/bias tiles carrying the valid_len mask (masked k rows get
bias=-1e6 -> exp underflows to 0; rows with valid_len==0 get scale=0,
bias=0 -> uniform attention, matching the reference). e is written in
fp16 so the attn@values and row-sum matmuls run at full PE rate; the
1/sum normalization is applied per-partition on the final PSUM->SBUF
copy.

All PE inputs (wq/wk, qT/kT, trig factors, e, values) are fp16:
1 cycle/row on the PE vs 4 for fp32, and half the DMA bytes.

Sharding: data-parallel over batch, 2 batches per core on 8 cores.
"""

import math
import sys

import numpy as np

sys.path.insert(0, "/opt/trn_rl_repo")

B, TQ, TK, DIN, H, DV = 16, 128, 256, 64, 256, 256
NCORES = 8
NB = B // NCORES
HB = 2  # h blocks of 128 partitions
NEG = -1000000.0
PI = math.pi
FX = 65536  # fixed-point phase resolution

# (fit range R, [alpha_m], [omega_m]) — offline weighted least squares with
# jointly optimized frequencies (Nelder-Mead over log omega).
FITS = [
    # R=4.25 mixed basis {x, x^3, sin w1 x, sin w2 x}: the polynomial part
    # factorizes into cheap PE channels (binomial expansion; pure-q terms
    # cancel in the softmax), leaving only TWO Sin evaluations on ACT.
    (4.25, [0.02924106, 0.20011121], [2.98939508, 1.61959803],
     0.57994674, -0.02086779),
    # R=4.25 M=3, weights tuned for end-to-end error on randn inputs
    (4.25, [1.14215814, 0.16947486, 0.02756034], [0.54450034, 1.69464501, 3.01720302]),
    # R=4.35 M=4 fit_max=2.65e-03
    (4.35, [1.15424910, 0.18579149, 0.03183397, 0.00486786], [0.51479938, 1.59132282, 2.77293209, 4.09736273]),
    # R=5.0 M=5 fit_max=1.15e-03
    (5.0, [1.17638592, 0.21722942, 0.04686229, 0.00903701, 0.00168118], [0.45910491, 1.40743013, 2.42317614, 3.51883170, 4.71208021]),
    # R=5.75 M=5 fit_max=2.69e-03
    (5.75, [1.19166771, 0.24153365, 0.06066643, 0.01419093, 0.00340621], [0.41816285, 1.27578909, 2.18259325, 3.14991184, 4.18200973]),
    # R=6.5 M=6 fit_max=2.18e-03
    (6.5, [1.22246911, 0.29583495, 0.09660808, 0.02984660, 0.00854484, 0.00257873], [0.33487488, 1.02337718, 1.75741117, 2.54559399, 3.38983334, 4.27904363]),
    # R=7.5 M=7 fit_max=2.83e-03
    (7.5, [1.23017784, 0.32207557, 0.13281313, 0.05605996, 0.02115424, 0.00734471, 0.00271091], [0.27799558, 0.82769422, 1.38567976, 1.99585889, 2.67159272, 3.40357483, 4.16577676]),
    # R=9.0 M=8 fit_max=3.42e-03
    (9.0, [1.23220768, 0.32660174, -0.09537761, 0.19814386, 0.04758848, 0.01766320, 0.00668698, 0.00282454], [0.29504675, 0.89849354, 1.31818698, 1.42932216, 2.09287727, 2.73674004, 3.39807058, 4.05802853]),
    # R=11.0 M=10 fit_max=2.34e-03
    (11.0, [1.25009981, 0.36156082, 0.16668645, 0.08288622, 0.04121021, 0.02000684, 0.00953954, 0.00468461, 0.00244755, -0.00000540], [0.21481795, 0.64601870, 1.08287839, 1.53102994, 1.99552922, 2.47865898, 2.98018571, 3.49646094, 4.00027643, 27.89010198]),
    # R=13.5 M=12 fit_max=1.65e-03
    (13.5, [1.25071458, 0.35628586, 0.11460491, 0.10187407, 0.14817974, -0.36003900, 0.32427418, 0.02462988, 0.01227283, 0.00633834, 0.00344839, 0.00201855], [0.19982948, 0.59368416, 0.91929799, 1.13521056, 1.56105869, 1.71511767, 1.75607839, 2.27997771, 2.72042538, 3.16077613, 3.60347782, 4.03539051]),
    # R=16.5 M=14 fit_max=2.77e-03
    (16.5, [1.25849188, 0.38272733, 0.19240301, 0.10549262, 0.02066639, 0.04720551, 0.04224601, -0.03774592, 0.04760619, 0.01225844, 0.00687190, 0.00399597, 0.01283338, -0.03588798, 0.02641497, 0.00090983, 0.00029855], [0.17064909, 0.51252204, 0.85576113, 1.19857872, 1.46425403, 1.59143819, 1.94973110, 2.13344383, 2.20286683, 2.61053434, 2.98029452, 3.35535440, 3.82535729, 3.89323910, 3.92054336, 4.37490434, 6.06447818]),
    # R=20.0 M=24 fit_max=8.08e-03
    (20.0, [1.26310001, 0.39699981, 0.50851750, -0.19663028, -0.15967006, 0.16325805, 0.07906020, 0.04867188, 0.03041830, 0.01901565, 0.01861664, -0.00714429, 0.00795889, 0.00512261, 0.00317850, 0.00126551, 0.00070946, 0.00065840, 0.00017538, -0.00000124, -0.00006095, -0.00011474, 0.00012987, -0.00002543], [0.14670123, 0.44208486, 0.76888929, 0.77629792, 0.84403806, 0.98925559, 1.30465383, 1.60553746, 1.90541734, 2.20846370, 2.53020022, 2.56316326, 2.81407758, 3.11334895, 3.41541978, 4.41970952, 4.63745933, 5.05356013, 6.39364912, 6.87284072, 7.81029508, 10.65968755, 10.76481947, 18.82746599]),
]

PQK = 2 * H + NB * TQ + NB * TK  # wq | qT | wk | kT   (fp16, 64 partitions)


def build_program(alpha, omega, exp_shift=0.0, c1=0.0, c3=0.0):
    """Build the per-core Bass program."""
    import concourse.bacc as bacc
    import concourse.bass as bass
    import concourse.mybir as mybir
    import concourse.tile as tile

    f32 = mybir.dt.float32
    f16 = mybir.dt.float16
    i32 = mybir.dt.int32
    i16 = mybir.dt.int16
    AF = mybir.ActivationFunctionType
    ALU = mybir.AluOpType

    M = len(alpha)
    mixed = c3 != 0.0
    CC = HB * M + (HB if mixed else 0)  # sine folds | 3c3*wv
    PQKX = PQK

    nc = bacc.Bacc("TRN2", target_bir_lowering=False, debug=False)

    QHALF = H + NB * TQ  # wq | qT
    pack64a_d = nc.dram_tensor("pack64a", [DIN, QHALF], f16, kind="ExternalInput").ap()
    pack64b_d = nc.dram_tensor("pack64b", [DIN, PQKX - QHALF], f16, kind="ExternalInput").ap()
    packc_d = nc.dram_tensor("packc", [128, CC], f32, kind="ExternalInput").ap()
    cmask_d = nc.dram_tensor("cmask", [1, NB * 2 * 128], f16, kind="ExternalInput").ap()
    packv_d = nc.dram_tensor("packv", [128, NB * 2 * DV], f16, kind="ExternalInput").ap()
    out_d = nc.dram_tensor("out", [NB, TQ, DV], f32, kind="ExternalOutput").ap()

    with tile.TileContext(nc) as tc:
        with (
            tc.tile_pool(name="singles", bufs=1) as singles,
            tc.tile_pool(name="trig", bufs=3) as trig,
            tc.tile_pool(name="soft", bufs=2) as soft,
            tc.tile_pool(name="pproj", bufs=1, space="PSUM") as pproj,
            tc.tile_pool(name="pscore", bufs=1, space="PSUM") as pscore,
            tc.tile_pool(name="ptail", bufs=1, space="PSUM") as ptail,
        ):
            # ---- constants / inputs to SBUF ----
            warm_t = singles.tile([128, 1], f32)
            nc.vector.memset(warm_t, 0.0)
            ones16 = singles.tile([128, 1], f16)
            nc.vector.memset(ones16, 1.0)
            zeros16 = singles.tile([128, 1], f16)
            nc.vector.memset(zeros16, 0.0)
            onesrow = singles.tile([1, TQ], f16)
            nc.vector.memset(onesrow, 1.0)
            bias_exp = singles.tile([128, 1], f32)
            nc.vector.memset(bias_exp, -float(exp_shift))

            pk64 = singles.tile([DIN, PQKX], f16)
            # q-half first: its (smaller) transfer completes sooner, so the
            # q projection and the scalar-engine copy chain start earlier
            nc.sync.dma_start(out=pk64[:, 0:QHALF], in_=pack64a_d)
            nc.sync.dma_start(out=pk64[:, QHALF:], in_=pack64b_d)
            pc = singles.tile([128, CC], f32)
            nc.sync.dma_start(out=pc, in_=packc_d)
            cmask = singles.tile([1, NB * 2 * 128], f16)
            nc.sync.dma_start(out=cmask, in_=cmask_d)
            pv = singles.tile([128, NB, 2, DV], f16)
            nc.sync.dma_start(
                out=pv,
                in_=bass.AP(tensor=packv_d.tensor, offset=0, ap=[[NB * 2 * DV, 128], [1, NB * 2 * DV]]),
            )

            wq_sb = pk64[:, 0:H]
            qTs = pk64[:, H : H + NB * TQ].rearrange("p (b x) -> p b x", b=NB)
            wk_sb = pk64[:, H + NB * TQ : 2 * H + NB * TQ]
            kTs = pk64[:, 2 * H + NB * TQ : PQK].rearrange(
                "p (b x) -> p b x", b=NB
            )
            fold_sb = pc[:, 0 : HB * M].rearrange("p (hb m) -> p hb m", hb=HB)

            # dummy Sin before the scalar-engine copies: pins the first
            # (startup-hidden) act-table load to the trig table so the Copy
            # activations below don't cause an extra mid-program load
            dummy_sin = singles.tile([128, 1], f16)
            nc.scalar.activation(
                out=dummy_sin, in_=warm_t, func=AF.Sin, bias=0.0, scale=1.0
            )

            # ---- projections (PE, fp16 in / fp32 psum): [h, hb, b, qi/ki] ----
            # tiny warm-up matmul first (into the qT region, re-zeroed by the
            # real projection's start): begins the PE p-state ramp early
            qT_ps = pproj.tile([128, HB, NB, TQ], f32)
            nc.tensor.matmul(
                qT_ps[0:1, 0, 0, 0:1], lhsT=warm_t, rhs=warm_t,
                start=True, stop=True, skip_group_check=True,
            )
            for hb in range(HB):
                nc.tensor.matmul(
                    qT_ps[:, hb, :, :].rearrange("p b x -> p (b x)"),
                    lhsT=wq_sb[:, hb * 128 : (hb + 1) * 128],
                    rhs=qTs.rearrange("p b x -> p (b x)"),
                    start=(hb == 0),
                    stop=(hb == HB - 1),
                )
            # kT_ps spans two 2KB psum zero regions (one per hb slice);
            # start/stop must bracket each region's writes separately.
            kT_ps = pproj.tile([128, HB, NB, TK], f32)
            for hb in range(HB):
                for b in range(NB):
                    nc.tensor.matmul(
                        kT_ps[:, hb, b, :],
                        lhsT=wk_sb[:, hb * 128 : (hb + 1) * 128],
                        rhs=kTs[:, b, :],
                        start=(b == 0),
                        stop=(b == NB - 1),
                    )

            # both PSUM->SBUF copies run on the (otherwise idle) scalar
            # engine so the DVE goes straight to the m=0 phase converts
            qTp = singles.tile([128, HB, NB, TQ], f32)
            nc.scalar.copy(out=qTp, in_=qT_ps)
            kTp = singles.tile([128, HB, NB, TK], f32)
            nc.scalar.copy(out=kTp, in_=kT_ps)

            # ---- transposed score accumulation over m sine terms ----
            # sT[b][k', kc, q] : k = kc*128 + k' on partitions. One 2KB psum
            # bank (= one zero region) per batch; the kc slices interleave
            # inside a single accumulation group bracketed by the batch's
            # first (start) and last (stop) matmul, like the baseline's
            # kT_ps hb regions.
            sT = [
                pscore.tile([128, 2, TQ], f32, tag=f"sT{b}", name=f"sT{b}")
                for b in range(NB)
            ]
            # per batch: 2 mask adds [+ 8 cubic channels] + sines
            n_mm = M * HB * 2 * 2 + 2 + (8 if mixed else 0)
            mm_i = [0] * NB
            sin_scale = 2.0 * PI / FX

            # additive softmax mask seeded into the score accumulators by
            # tiny 1-partition matmuls (c[k] x ones[q]) while the PE is
            # otherwise idle; runs right after the input DMA lands
            for b in range(NB):
                for kc in range(2):
                    r0 = (b * 2 + kc) * 128
                    nc.tensor.matmul(
                        sT[b][:, kc, :],
                        lhsT=cmask[0:1, r0 : r0 + 128],
                        rhs=onesrow,
                        start=(mm_i[b] == 0),
                        stop=False,
                    )
                    mm_i[b] += 1

            # ---- polynomial part of the mixed tanh basis ----
            # c1*x + c3*x^3 with x = q+k expands binomially: pure-q terms
            # cancel in the softmax, the pure-k rank-1 (c1*k + c3*k^3 through
            # wv) is folded into the cmask rows on the HOST, and only the
            # cross channels 3c3*(q^2 k + q k^2) run here as f16 PE matmuls.
            if mixed:
                f3a = pc[:, HB * M : HB * M + HB]  # 3*c3*wv per hb
                q2 = singles.tile([128, HB, NB, TQ], f32)
                k2 = singles.tile([128, HB, NB, TK], f32)
                kp16 = singles.tile([128, HB, NB, TK], f16)
                k2_16 = singles.tile([128, HB, NB, TK], f16)
                fq2 = singles.tile([128, HB, NB, TQ], f16)
                fq1 = singles.tile([128, HB, NB, TQ], f16)

            def emit_poly_powers():
                # q square on gpsimd (ready before the m0 AC folds need
                # Pool); k square on the DVE after the m0 converts
                nc.gpsimd.tensor_tensor(out=q2, in0=qTp, in1=qTp, op=ALU.mult)

            def emit_poly_folds():
                for hb in range(HB):
                    nc.gpsimd.tensor_scalar(
                        out=fq2[:, hb], in0=q2[:, hb],
                        scalar1=f3a[:, hb : hb + 1], scalar2=None, op0=ALU.mult,
                    )
                    nc.gpsimd.tensor_scalar(
                        out=fq1[:, hb], in0=qTp[:, hb],
                        scalar1=f3a[:, hb : hb + 1], scalar2=None, op0=ALU.mult,
                    )
                # k^2 squared directly in f16 (2x DVE mode, and it skips the
                # slow 1x f32 TensorTensor) so the poly channel matmuls are
                # ready BEFORE the last harmonic's burst window
                nc.vector.tensor_copy(out=kp16, in_=kTp)
                nc.vector.tensor_tensor(out=k2_16, in0=kp16, in1=kp16, op=ALU.mult)

            def emit_poly_matmuls():
                for b in range(NB):
                    for kc in range(2):
                        ks = slice(kc * 128, (kc + 1) * 128)
                        for hb in range(HB):
                            nc.tensor.matmul(
                                sT[b][:, kc, :],
                                lhsT=kp16[:, hb, b, ks],
                                rhs=fq2[:, hb, b, :],
                                start=False, stop=False,
                            )
                            mm_i[b] += 1
                            nc.tensor.matmul(
                                sT[b][:, kc, :],
                                lhsT=k2_16[:, hb, b, ks],
                                rhs=fq1[:, hb, b, :],
                                start=False, stop=False,
                            )
                            mm_i[b] += 1

            def emit_phase(m):
                """DVE int phase converts + ACT Sin for sine term m.

                Phase tiles hold [hb, sin|cos, b, x] so one ACT Sin per side
                sweeps both quadratures. ACT reads the LOW int16 half of each
                int32 via a bitcast + stride-2 AP; the SIGNED int16 view puts
                the phase in [-pi, pi) directly."""
                w16 = float(np.float32(omega[m] / (2.0 * PI) * FX))
                nq = trig.tile([128, HB, 2, NB, TQ], i32, tag="nq", name="nq")
                nc.vector.tensor_scalar(
                    out=nq[:, :, 0], in0=qTp, scalar1=w16, scalar2=None, op0=ALU.mult
                )
                nc.vector.tensor_scalar(
                    out=nq[:, :, 1], in0=qTp, scalar1=w16, scalar2=float(FX // 4),
                    op0=ALU.mult, op1=ALU.add,
                )
                nk = trig.tile([128, HB, 2, NB, TK], i32, tag="nk", name="nk")
                nc.vector.tensor_scalar(
                    out=nk[:, :, 0], in0=kTp, scalar1=w16, scalar2=None, op0=ALU.mult
                )
                nc.vector.tensor_scalar(
                    out=nk[:, :, 1], in0=kTp, scalar1=w16, scalar2=float(FX // 4),
                    op0=ALU.mult, op1=ALU.add,
                )
                # t[:,hb,0] = sin(w x), t[:,hb,1] = cos(w x)
                tq = trig.tile([128, HB, 2, NB, TQ], f16, tag="tq", name="tq")
                nc.scalar.activation(
                    out=tq, in_=nq.bitcast(i16)[:, :, :, :, 0::2], func=AF.Sin,
                    bias=0.0, scale=sin_scale,
                )
                tk = trig.tile([128, HB, 2, NB, TK], f16, tag="tk", name="tk")
                nc.scalar.activation(
                    out=tk, in_=nk.bitcast(i16)[:, :, :, :, 0::2], func=AF.Sin,
                    bias=0.0, scale=sin_scale,
                )
                # zero-contribution keep-warm matmuls, one per fresh trig
                # tile: they space out through the mainloop and stop the PE
                # p-state ramp from resetting during its long idle, so the
                # final matmul burst runs at full clock. Skipped for the
                # last term so they don't steal the burst's first slot.
                if m < M - 1:
                    for rhs in (tq[:, 0, 0, 0, 0:1], tk[:, 0, 0, 0, 0:1]):
                        nc.tensor.matmul(
                            sT[0][0:1, 0, 0:1], lhsT=zeros16, rhs=rhs,
                            start=False, stop=False, skip_group_check=True,
                        )
                return tq, tk

            def emit_reduce(m, tq, tk):
                """Fold alpha_m*wv into the q side, then accumulate the
                transposed scores: sT[b][kc] += tkc.T @ (fold*sin q)
                + tks.T @ (fold*cos q)."""
                AC = trig.tile([128, HB, 2, NB, TQ], f16, tag="AC", name="AC")
                for hb in range(HB):
                    nc.gpsimd.tensor_scalar(
                        out=AC[:, hb], in0=tq[:, hb],
                        scalar1=fold_sb[:, hb, m : m + 1], scalar2=None, op0=ALU.mult,
                    )
                for b in range(NB):
                    for kc in range(2):
                        for hb in range(HB):
                            ks = slice(kc * 128, (kc + 1) * 128)
                            nc.tensor.matmul(
                                sT[b][:, kc, :],
                                lhsT=tk[:, hb, 1, b, ks],
                                rhs=AC[:, hb, 0, b, :],
                                start=(mm_i[b] == 0),
                                stop=(mm_i[b] == n_mm - 1),
                            )
                            mm_i[b] += 1
                            nc.tensor.matmul(
                                sT[b][:, kc, :],
                                lhsT=tk[:, hb, 0, b, ks],
                                rhs=AC[:, hb, 1, b, :],
                                start=(mm_i[b] == 0),
                                stop=(mm_i[b] == n_mm - 1),
                            )
                            mm_i[b] += 1

            # software-pipeline: folds+matmuls for term m are emitted after
            # phase m+1, so gpsimd's AC(m) doesn't stall the DVE->ACT chain.
            # The polynomial pieces slot in behind the m=0 emissions.
            pending = None
            for m in range(M):
                cur = emit_phase(m)
                if mixed and m == 0:
                    emit_poly_powers()
                if pending is not None:
                    emit_reduce(*pending)
                    if mixed and pending[0] == 0:
                        emit_poly_folds()
                        emit_poly_matmuls()
                pending = (m, *cur)
            emit_reduce(*pending)

            # ---- masked softmax + attn @ values, per batch ----
            # The mask lives entirely in the Exp activation's per-partition
            # scale/bias (k is the partition dim of sT): masked rows get
            # bias=-1e6 (exp -> exactly 0), valid_len==0 batches get
            # scale=0,bias=0 (uniform attention). exp_shift is folded into
            # bias host-side. Row sums over k via a ones-matmul; the 1/sum
            # is applied per-partition (q) on the PSUM->SBUF output copy.
            out_sb = soft.tile([128, NB, DV], f32, tag="out_sb", name="out_sb")
            e16 = soft.tile([128, NB, 2, TQ], f16, tag="e16", name="e16")
            for b in range(NB):
                nc.scalar.activation(
                    out=e16[:, b], in_=sT[b], func=AF.Exp,
                    bias=bias_exp, scale=1.0,
                )
            # per-batch accumulator tile: value columns + a sums column in
            # one psum bank, so each batch's group stops at its OWN last
            # matmul and its reciprocal/normalize isn't gated on the other
            # batch. Order per batch: value kc0 (start) ... sums kc1 (stop).
            tails = [
                ptail.tile([128, DV + 1], f32, tag=f"tail{b}", name=f"tail{b}")
                for b in range(NB)
            ]
            for b in range(NB):
                for kc in range(2):
                    nc.tensor.matmul(
                        tails[b][:, 0:DV],
                        lhsT=e16[:, b, kc, :],
                        rhs=pv[:, b, kc, :],
                        start=(kc == 0),
                        stop=False,
                    )
                    nc.tensor.matmul(
                        tails[b][:, DV : DV + 1],
                        lhsT=e16[:, b, kc, :],
                        rhs=ones16,
                        start=False,
                        stop=(kc == 1),
                    )
            # b1's normalize runs on the scalar engine (Identity with a
            # per-partition scale) so it isn't queued behind b0's on DVE;
            # separate out tiles avoid a cross-engine whole-tile WAW stall.
            rr = soft.tile([128, NB], f32, tag="r", name="r")
            nc.vector.reciprocal(out=rr[:, 0:1], in_=tails[0][:, DV : DV + 1])
            nc.vector.tensor_scalar(
                out=out_sb[:, 0, :], in0=tails[0][:, 0:DV], scalar1=rr[:, 0:1],
                scalar2=None, op0=ALU.mult,
            )
            nc.sync.dma_start(out=out_d[0], in_=out_sb[:, 0, :])
            out_sb1 = soft.tile([128, DV], f32, tag="out_sb1", name="out_sb1")
            nc.vector.reciprocal(out=rr[:, 1:2], in_=tails[1][:, DV : DV + 1])
            nc.vector.tensor_scalar(
                out=out_sb1, in0=tails[1][:, 0:DV], scalar1=rr[:, 1:2],
                scalar2=None, op0=ALU.mult,
            )
            nc.sync.dma_start(out=out_d[1], in_=out_sb1)

    nc.compile()
    return nc


def prepare_in_maps(queries, keys, values, valid_lens, wq, wk, wv, alpha,
                    exp_shift, c1=0.0, c3=0.0):
    """Host-side sharding + layout transforms. Returns list of 8 input dicts."""
    M = len(alpha)
    mixed = c3 != 0.0
    queries = np.ascontiguousarray(queries, dtype=np.float32)
    keys = np.ascontiguousarray(keys, dtype=np.float32)
    values = np.ascontiguousarray(values, dtype=np.float32)
    wq16 = np.ascontiguousarray(wq, dtype=np.float16)
    wk16 = np.ascontiguousarray(wk, dtype=np.float16)
    wv = np.asarray(wv, dtype=np.float32)
    valid_lens = np.asarray(valid_lens)

    # fold[p, hb, m] = alpha_m * wv[hb*128 + p]
    fold = np.empty((128, HB, M), np.float32)
    for hb in range(HB):
        fold[:, hb, :] = np.asarray(alpha, np.float64)[None, :] * wv[
            hb * 128 : (hb + 1) * 128, None
        ]

    CNEG = -60000.0  # f16-representable; exp underflows to exactly 0
    karange = np.arange(128)
    in_maps = []
    for c in range(NCORES):
        bs = slice(c * NB, (c + 1) * NB)
        qT = queries[bs].transpose(2, 0, 1).reshape(DIN, NB, TQ).copy()
        kT = keys[bs].transpose(2, 0, 1).reshape(DIN, NB, TK).copy()
        # additive mask rows per (b, kchunk); valid_len==0 batches get
        # zeroed q/k (scores==0 exactly) + zero mask -> uniform attention
        cmask = np.zeros((NB, 2, 128), np.float16)
        for j, vl in enumerate(valid_lens[bs]):
            vl = int(vl)
            if vl <= 0:
                qT[:, j] = 0.0
                kT[:, j] = 0.0
            else:
                for kc in range(2):
                    cmask[j, kc] = np.where(
                        (kc * 128 + karange) < vl, 0.0, CNEG
                    ).astype(np.float16)
        p64 = [wq16, qT.reshape(DIN, NB * TQ).astype(np.float16), wk16,
               kT.reshape(DIN, NB * TK).astype(np.float16)]
        pcc = [fold.reshape(128, HB * M)]
        if mixed:
            f3a = np.empty((128, HB), np.float32)
            for hb in range(HB):
                f3a[:, hb] = 3.0 * c3 * wv[hb * 128 : (hb + 1) * 128]
            pcc += [f3a]
            # pure-k part of the polynomial, precontracted with wv on the
            # host: vk[k] = sum_h wv_h * (c1*kp + c3*kp^3); folded into the
            # additive cmask rows (zeroed-key batches get vk == 0)
            kp = kT.reshape(DIN, NB * TK).T.astype(np.float64) @ wk.astype(
                np.float64
            )
            vk = (c1 * kp + c3 * kp**3) @ wv.astype(np.float64)
            cmask += vk.reshape(NB, 2, 128).astype(np.float16)
        pack64a = np.concatenate(p64[:2], axis=1)
        pack64b = np.concatenate(p64[2:], axis=1)
        vals = values[bs].reshape(NB, 2, 128, DV).transpose(2, 0, 1, 3)
        packv = vals.reshape(128, NB * 2 * DV).astype(np.float16)
        packc = np.concatenate(pcc, axis=1).astype(np.float32)
        in_maps.append(
            {
                "pack64a": np.ascontiguousarray(pack64a),
                "pack64b": np.ascontiguousarray(pack64b),
                "packc": np.ascontiguousarray(packc),
                "cmask": np.ascontiguousarray(cmask.reshape(1, NB * 2 * 128)),
                "packv": np.ascontiguousarray(packv),
            }
        )
    return in_maps


def _pick_fit(queries, keys, wq, wk):
    q = queries.reshape(-1, DIN).astype(np.float32) @ wq.astype(np.float32)
    k = keys.reshape(-1, DIN).astype(np.float32) @ wk.astype(np.float32)
    qb = q.reshape(B, TQ, H)
    kb = k.reshape(B, TK, H)
    hi = (qb.max(1) + kb.max(1)).max()
    lo = (qb.min(1) + kb.min(1)).min()
    r_needed = max(abs(hi), abs(lo))
    for ent in FITS:
        if ent[0] >= r_needed + 0.05:
            break
    else:
        ent = FITS[-1]
    R, alpha, omega = ent[0], ent[1], ent[2]
    c1 = ent[3] if len(ent) > 3 else 0.0
    c3 = ent[4] if len(ent) > 4 else 0.0
    return R, alpha, omega, c1, c3


_prog_cache = {}


def kernel(queries, keys, values, valid_lens, wq, wk, wv):
    from concourse import bass_utils

    queries = np.asarray(queries)
    keys = np.asarray(keys)
    values = np.asarray(values)
    valid_lens = np.asarray(valid_lens)
    wq = np.asarray(wq)
    wk = np.asarray(wk)
    wv = np.asarray(wv)

    R, alpha, omega, c1, c3 = _pick_fit(queries, keys, wq, wk)
    # scores bounded by ~sum|wv| * max|approx tanh|; keep exp(score) within
    # fp16 range (e is stored as fp16)
    bound = float(np.abs(wv).sum()) * 1.01
    exp_shift = max(0.0, bound - 10.0)

    key = (R, len(alpha), round(exp_shift, 3))
    if key not in _prog_cache:
        _prog_cache[key] = build_program(alpha, omega, exp_shift, c1, c3)
    nc = _prog_cache[key]

    in_maps = prepare_in_maps(
        queries, keys, values, valid_lens, wq, wk, wv, alpha, exp_shift, c1, c3
    )
    res = bass_utils.run_bass_kernel_spmd(nc, in_maps, core_ids=list(range(NCORES)))
    out = np.concatenate([r["out"] for r in res.results], axis=0)
    return out.astype(np.float32)


if __name__ == "__main__":
    rng = np.random.default_rng(0)
    inputs = {
        "queries": rng.standard_normal((B, TQ, DIN), dtype=np.float32),
        "keys": rng.standard_normal((B, TK, DIN), dtype=np.float32),
        "values": rng.standard_normal((B, TK, DV), dtype=np.float32),
        "valid_lens": rng.integers(0, TK, size=(B,)).astype(np.int32),
        "wq": (rng.standard_normal((DIN, H), dtype=np.float32) * 0.05),
        "wk": (rng.standard_normal((DIN, H), dtype=np.float32) * 0.05),
        "wv": (rng.standard_normal((H,), dtype=np.float32) * 0.05),
    }
    out = kernel(**inputs)
    print("out", out.shape, out.dtype)


# revision 80
# speedup vs baseline: 1.0257x; 1.0038x over previous
"""Bahdanau attention Trainium2 kernel.

Math: out = softmax_k(mask(score)) @ values with
  score[b,q,k] = sum_h wv[h] * tanh(Q[b,q,h] + K[b,k,h]),
  Q = queries @ wq, K = keys @ wk.

tanh(x) is approximated by a mixed polynomial + free-frequency sine
basis
  tanh(x) ~= c1*x + c3*x^3 + sum_m alpha_m sin(omega_m x)
(coefficients and omegas jointly optimized per fit range offline,
hardcoded below). Each sine term factorizes through
sin(omega(q+k)) = sin(wq)cos(wk) + cos(wq)sin(wk) into dense
[Tq,H]x[H,Tk] matmuls on the PE array; the polynomial part expands
binomially: pure-q terms are constant along the softmax axis and
cancel, the pure-k rank-1 part (c1*k + c3*k^3 contracted with wv) is
precomputed on the host and folded into the additive-mask rows, and
only the cross terms 3c3*(q^2 k + q k^2) run on device as fp16 PE
matmul channels. This leaves just TWO Sin evaluations on the
activation engine, which is the critical resource.

The scalar engine's Sin is only valid on [-pi, pi], so arguments are
range-reduced in 16-bit fixed point using the HW's round-to-nearest
fp32->int32 conversion:
  n = round(x * omega/(2pi) * 65536)     (one tensor_scalar, int32 out)
ACT then reads the LOW int16 half of each int32 via a bitcast +
stride-2 AP; the SIGNED int16 view puts the phase in [-pi, pi)
directly (sin arg = v16 * 2pi/65536, no bias).

Scores are accumulated TRANSPOSED (scoresT[k, q], k on partitions, two
128-row chunks per batch) by swapping matmul lhsT/rhs. This removes the
tail transpose entirely and lets the masked softmax fold completely into
the Exp activation: exp(score*scale + bias) with per-partition
scale/bias tiles carrying the valid_len mask (masked k rows get
bias=-1e6 -> exp underflows to 0; rows with valid_len==0 get scale=0,
bias=0 -> uniform attention, matching the reference). e is written in
fp16 so the attn@values and row-sum matmuls run at full PE rate; the
1/sum normalization is applied per-partition on the final PSUM->SBUF
copy.

All PE inputs (wq/wk, qT/kT, trig factors, e, values) are fp16:
1 cycle/row on the PE vs 4 for fp32, and half the DMA bytes.

Sharding: data-parallel over batch, 2 batches per core on 8 cores.
"""

import math
import sys

import numpy as np

sys.path.insert(0, "/opt/trn_rl_repo")

B, TQ, TK, DIN, H, DV = 16, 128, 256, 64, 256, 256
NCORES = 8
NB = B // NCORES
HB = 2  # h blocks of 128 partitions
NEG = -1000000.0
PI = math.pi
FX = 65536  # fixed-point phase resolution

# (fit range R, [alpha_m], [omega_m]) — offline weighted least squares with
# jointly optimized frequencies (Nelder-Mead over log omega).
FITS = [
    # R=4.25 mixed basis {x, x^3, sin w1 x, sin w2 x}: the polynomial part
    # factorizes into cheap PE channels (binomial expansion; pure-q terms
    # cancel in the softmax), leaving only TWO Sin evaluations on ACT.
    (4.25, [0.02924106, 0.20011121], [2.98939508, 1.61959803],
     0.57994674, -0.02086779),
    # R=4.25 M=3, weights tuned for end-to-end error on randn inputs
    (4.25, [1.14215814, 0.16947486, 0.02756034], [0.54450034, 1.69464501, 3.01720302]),
    # R=4.35 M=4 fit_max=2.65e-03
    (4.35, [1.15424910, 0.18579149, 0.03183397, 0.00486786], [0.51479938, 1.59132282, 2.77293209, 4.09736273]),
    # R=5.0 M=5 fit_max=1.15e-03
    (5.0, [1.17638592, 0.21722942, 0.04686229, 0.00903701, 0.00168118], [0.45910491, 1.40743013, 2.42317614, 3.51883170, 4.71208021]),
    # R=5.75 M=5 fit_max=2.69e-03
    (5.75, [1.19166771, 0.24153365, 0.06066643, 0.01419093, 0.00340621], [0.41816285, 1.27578909, 2.18259325, 3.14991184, 4.18200973]),
    # R=6.5 M=6 fit_max=2.18e-03
    (6.5, [1.22246911, 0.29583495, 0.09660808, 0.02984660, 0.00854484, 0.00257873], [0.33487488, 1.02337718, 1.75741117, 2.54559399, 3.38983334, 4.27904363]),
    # R=7.5 M=7 fit_max=2.83e-03
    (7.5, [1.23017784, 0.32207557, 0.13281313, 0.05605996, 0.02115424, 0.00734471, 0.00271091], [0.27799558, 0.82769422, 1.38567976, 1.99585889, 2.67159272, 3.40357483, 4.16577676]),
    # R=9.0 M=8 fit_max=3.42e-03
    (9.0, [1.23220768, 0.32660174, -0.09537761, 0.19814386, 0.04758848, 0.01766320, 0.00668698, 0.00282454], [0.29504675, 0.89849354, 1.31818698, 1.42932216, 2.09287727, 2.73674004, 3.39807058, 4.05802853]),
    # R=11.0 M=10 fit_max=2.34e-03
    (11.0, [1.25009981, 0.36156082, 0.16668645, 0.08288622, 0.04121021, 0.02000684, 0.00953954, 0.00468461, 0.00244755, -0.00000540], [0.21481795, 0.64601870, 1.08287839, 1.53102994, 1.99552922, 2.47865898, 2.98018571, 3.49646094, 4.00027643, 27.89010198]),
    # R=13.5 M=12 fit_max=1.65e-03
    (13.5, [1.25071458, 0.35628586, 0.11460491, 0.10187407, 0.14817974, -0.36003900, 0.32427418, 0.02462988, 0.01227283, 0.00633834, 0.00344839, 0.00201855], [0.19982948, 0.59368416, 0.91929799, 1.13521056, 1.56105869, 1.71511767, 1.75607839, 2.27997771, 2.72042538, 3.16077613, 3.60347782, 4.03539051]),
    # R=16.5 M=14 fit_max=2.77e-03
    (16.5, [1.25849188, 0.38272733, 0.19240301, 0.10549262, 0.02066639, 0.04720551, 0.04224601, -0.03774592, 0.04760619, 0.01225844, 0.00687190, 0.00399597, 0.01283338, -0.03588798, 0.02641497, 0.00090983, 0.00029855], [0.17064909, 0.51252204, 0.85576113, 1.19857872, 1.46425403, 1.59143819, 1.94973110, 2.13344383, 2.20286683, 2.61053434, 2.98029452, 3.35535440, 3.82535729, 3.89323910, 3.92054336, 4.37490434, 6.06447818]),
    # R=20.0 M=24 fit_max=8.08e-03
    (20.0, [1.26310001, 0.39699981, 0.50851750, -0.19663028, -0.15967006, 0.16325805, 0.07906020, 0.04867188, 0.03041830, 0.01901565, 0.01861664, -0.00714429, 0.00795889, 0.00512261, 0.00317850, 0.00126551, 0.00070946, 0.00065840, 0.00017538, -0.00000124, -0.00006095, -0.00011474, 0.00012987, -0.00002543], [0.14670123, 0.44208486, 0.76888929, 0.77629792, 0.84403806, 0.98925559, 1.30465383, 1.60553746, 1.90541734, 2.20846370, 2.53020022, 2.56316326, 2.81407758, 3.11334895, 3.41541978, 4.41970952, 4.63745933, 5.05356013, 6.39364912, 6.87284072, 7.81029508, 10.65968755, 10.76481947, 18.82746599]),
]

PQK = 2 * H + NB * TQ + NB * TK  # wq | qT | wk | kT   (fp16, 64 partitions)


def build_program(alpha, omega, exp_shift=0.0, c1=0.0, c3=0.0):
    """Build the per-core Bass program."""
    import concourse.bacc as bacc
    import concourse.bass as bass
    import concourse.mybir as mybir
    import concourse.tile as tile

    f32 = mybir.dt.float32
    f16 = mybir.dt.float16
    i32 = mybir.dt.int32
    i16 = mybir.dt.int16
    AF = mybir.ActivationFunctionType
    ALU = mybir.AluOpType

    M = len(alpha)
    mixed = c3 != 0.0
    CC = HB * M + (HB if mixed else 0)  # sine folds | 3c3*wv
    PQKX = PQK

    nc = bacc.Bacc("TRN2", target_bir_lowering=False, debug=False)

    QHALF = H + NB * TQ  # wq | qT
    pack64a_d = nc.dram_tensor("pack64a", [DIN, QHALF], f16, kind="ExternalInput").ap()
    pack64b_d = nc.dram_tensor("pack64b", [DIN, PQKX - QHALF], f16, kind="ExternalInput").ap()
    packc_d = nc.dram_tensor("packc", [128, CC], f32, kind="ExternalInput").ap()
    cmask_d = nc.dram_tensor("cmask", [1, NB * 2 * 128], f16, kind="ExternalInput").ap()
    packv_d = nc.dram_tensor("packv", [128, NB * 2 * DV], f16, kind="ExternalInput").ap()
    out_d = nc.dram_tensor("out", [NB, TQ, DV], f32, kind="ExternalOutput").ap()

    with tile.TileContext(nc) as tc:
        with (
            tc.tile_pool(name="singles", bufs=1) as singles,
            tc.tile_pool(name="trig", bufs=3) as trig,
            tc.tile_pool(name="soft", bufs=2) as soft,
            tc.tile_pool(name="pproj", bufs=1, space="PSUM") as pproj,
            tc.tile_pool(name="pscore", bufs=1, space="PSUM") as pscore,
            tc.tile_pool(name="ptail", bufs=1, space="PSUM") as ptail,
        ):
            # ---- constants / inputs to SBUF ----
            warm_t = singles.tile([128, 1], f32)
            nc.vector.memset(warm_t, 0.0)
            ones16 = singles.tile([128, 1], f16)
            nc.vector.memset(ones16, 1.0)
            zeros16 = singles.tile([128, 1], f16)
            nc.vector.memset(zeros16, 0.0)
            onesrow = singles.tile([1, TQ], f16)
            nc.vector.memset(onesrow, 1.0)
            bias_exp = singles.tile([128, 1], f32)
            nc.vector.memset(bias_exp, -float(exp_shift))

            pk64 = singles.tile([DIN, PQKX], f16)
            # q-half first: its (smaller) transfer completes sooner, so the
            # q projection and the scalar-engine copy chain start earlier
            nc.sync.dma_start(out=pk64[:, 0:QHALF], in_=pack64a_d)
            nc.sync.dma_start(out=pk64[:, QHALF:], in_=pack64b_d)
            pc = singles.tile([128, CC], f32)
            nc.sync.dma_start(out=pc, in_=packc_d)
            cmask = singles.tile([1, NB * 2 * 128], f16)
            nc.sync.dma_start(out=cmask, in_=cmask_d)
            pv = singles.tile([128, NB, 2, DV], f16)
            nc.sync.dma_start(
                out=pv,
                in_=bass.AP(tensor=packv_d.tensor, offset=0, ap=[[NB * 2 * DV, 128], [1, NB * 2 * DV]]),
            )

            wq_sb = pk64[:, 0:H]
            qTs = pk64[:, H : H + NB * TQ].rearrange("p (b x) -> p b x", b=NB)
            wk_sb = pk64[:, H + NB * TQ : 2 * H + NB * TQ]
            kTs = pk64[:, 2 * H + NB * TQ : PQK].rearrange(
                "p (b x) -> p b x", b=NB
            )
            fold_sb = pc[:, 0 : HB * M].rearrange("p (hb m) -> p hb m", hb=HB)

            # dummy Sin before the scalar-engine copies: pins the first
            # (startup-hidden) act-table load to the trig table so the Copy
            # activations below don't cause an extra mid-program load
            dummy_sin = singles.tile([128, 1], f16)
            nc.scalar.activation(
                out=dummy_sin, in_=warm_t, func=AF.Sin, bias=0.0, scale=1.0
            )

            # ---- projections (PE, fp16 in / fp32 psum): [h, hb, b, qi/ki] ----
            # tiny warm-up matmul first (into the qT region, re-zeroed by the
            # real projection's start): begins the PE p-state ramp early
            qT_ps = pproj.tile([128, HB, NB, TQ], f32)
            nc.tensor.matmul(
                qT_ps[0:1, 0, 0, 0:1], lhsT=warm_t, rhs=warm_t,
                start=True, stop=True, skip_group_check=True,
            )
            for hb in range(HB):
                nc.tensor.matmul(
                    qT_ps[:, hb, :, :].rearrange("p b x -> p (b x)"),
                    lhsT=wq_sb[:, hb * 128 : (hb + 1) * 128],
                    rhs=qTs.rearrange("p b x -> p (b x)"),
                    start=(hb == 0),
                    stop=(hb == HB - 1),
                )
            # kT_ps spans two 2KB psum zero regions (one per hb slice); one
            # 512-col matmul per region covers both batches (the kT columns
            # are contiguous in pk64), halving the dispatch count on the
            # chain that feeds the scalar-engine kTp copy.
            kT_ps = pproj.tile([128, HB, NB, TK], f32)
            for hb in range(HB):
                nc.tensor.matmul(
                    kT_ps[:, hb].rearrange("p b x -> p (b x)"),
                    lhsT=wk_sb[:, hb * 128 : (hb + 1) * 128],
                    rhs=pk64[:, 2 * H + NB * TQ : PQK],
                    start=True,
                    stop=True,
                )

            # both PSUM->SBUF copies run on the (otherwise idle) scalar
            # engine so the DVE goes straight to the m=0 phase converts
            qTp = singles.tile([128, HB, NB, TQ], f32)
            nc.scalar.copy(out=qTp, in_=qT_ps)
            kTp = singles.tile([128, HB, NB, TK], f32)
            nc.scalar.copy(out=kTp, in_=kT_ps)

            # ---- transposed score accumulation over m sine terms ----
            # sT[b][k', kc, q] : k = kc*128 + k' on partitions. One 2KB psum
            # bank (= one zero region) per batch; the kc slices interleave
            # inside a single accumulation group bracketed by the batch's
            # first (start) and last (stop) matmul, like the baseline's
            # kT_ps hb regions.
            sT = [
                pscore.tile([128, 2, TQ], f32, tag=f"sT{b}", name=f"sT{b}")
                for b in range(NB)
            ]
            # per batch: 2 mask adds [+ 8 cubic channels] + sines
            n_mm = M * HB * 2 * 2 + 2 + (8 if mixed else 0)
            mm_i = [0] * NB
            sin_scale = 2.0 * PI / FX

            # additive softmax mask seeded into the score accumulators by
            # tiny 1-partition matmuls (c[k] x ones[q]) while the PE is
            # otherwise idle; runs right after the input DMA lands
            for b in range(NB):
                for kc in range(2):
                    r0 = (b * 2 + kc) * 128
                    nc.tensor.matmul(
                        sT[b][:, kc, :],
                        lhsT=cmask[0:1, r0 : r0 + 128],
                        rhs=onesrow,
                        start=(mm_i[b] == 0),
                        stop=False,
                    )
                    mm_i[b] += 1

            # ---- polynomial part of the mixed tanh basis ----
            # c1*x + c3*x^3 with x = q+k expands binomially: pure-q terms
            # cancel in the softmax, the pure-k rank-1 (c1*k + c3*k^3 through
            # wv) is folded into the cmask rows on the HOST, and only the
            # cross channels 3c3*(q^2 k + q k^2) run here as f16 PE matmuls.
            if mixed:
                f3a = pc[:, HB * M : HB * M + HB]  # 3*c3*wv per hb
                q2 = singles.tile([128, HB, NB, TQ], f32)
                k2 = singles.tile([128, HB, NB, TK], f32)
                kp16 = singles.tile([128, HB, NB, TK], f16)
                k2_16 = singles.tile([128, HB, NB, TK], f16)
                fq2 = singles.tile([128, HB, NB, TQ], f16)
                fq1 = singles.tile([128, HB, NB, TQ], f16)

            def emit_poly_powers():
                # q square on gpsimd (ready before the m0 AC folds need
                # Pool); k square on the DVE after the m0 converts
                nc.gpsimd.tensor_tensor(out=q2, in0=qTp, in1=qTp, op=ALU.mult)

            def emit_poly_folds():
                for hb in range(HB):
                    nc.gpsimd.tensor_scalar(
                        out=fq2[:, hb], in0=q2[:, hb],
                        scalar1=f3a[:, hb : hb + 1], scalar2=None, op0=ALU.mult,
                    )
                    nc.gpsimd.tensor_scalar(
                        out=fq1[:, hb], in0=qTp[:, hb],
                        scalar1=f3a[:, hb : hb + 1], scalar2=None, op0=ALU.mult,
                    )
                # k^2 squared directly in f16 (2x DVE mode, and it skips the
                # slow 1x f32 TensorTensor) so the poly channel matmuls are
                # ready BEFORE the last harmonic's burst window
                nc.vector.tensor_copy(out=kp16, in_=kTp)
                nc.vector.tensor_tensor(out=k2_16, in0=kp16, in1=kp16, op=ALU.mult)

            def emit_poly_matmuls():
                for b in range(NB):
                    for kc in range(2):
                        ks = slice(kc * 128, (kc + 1) * 128)
                        for hb in range(HB):
                            nc.tensor.matmul(
                                sT[b][:, kc, :],
                                lhsT=kp16[:, hb, b, ks],
                                rhs=fq2[:, hb, b, :],
                                start=False, stop=False,
                            )
                            mm_i[b] += 1
                            nc.tensor.matmul(
                                sT[b][:, kc, :],
                                lhsT=k2_16[:, hb, b, ks],
                                rhs=fq1[:, hb, b, :],
                                start=False, stop=False,
                            )
                            mm_i[b] += 1

            def emit_phase(m):
                """DVE int phase converts + ACT Sin for sine term m.

                Phase tiles hold [hb, sin|cos, b, x] so one ACT Sin per side
                sweeps both quadratures. ACT reads the LOW int16 half of each
                int32 via a bitcast + stride-2 AP; the SIGNED int16 view puts
                the phase in [-pi, pi) directly."""
                w16 = float(np.float32(omega[m] / (2.0 * PI) * FX))
                nq = trig.tile([128, HB, 2, NB, TQ], i32, tag="nq", name="nq")
                nc.vector.tensor_scalar(
                    out=nq[:, :, 0], in0=qTp, scalar1=w16, scalar2=None, op0=ALU.mult
                )
                nc.vector.tensor_scalar(
                    out=nq[:, :, 1], in0=qTp, scalar1=w16, scalar2=float(FX // 4),
                    op0=ALU.mult, op1=ALU.add,
                )
                nk = trig.tile([128, HB, 2, NB, TK], i32, tag="nk", name="nk")
                nc.vector.tensor_scalar(
                    out=nk[:, :, 0], in0=kTp, scalar1=w16, scalar2=None, op0=ALU.mult
                )
                nc.vector.tensor_scalar(
                    out=nk[:, :, 1], in0=kTp, scalar1=w16, scalar2=float(FX // 4),
                    op0=ALU.mult, op1=ALU.add,
                )
                # t[:,hb,0] = sin(w x), t[:,hb,1] = cos(w x)
                tq = trig.tile([128, HB, 2, NB, TQ], f16, tag="tq", name="tq")
                nc.scalar.activation(
                    out=tq, in_=nq.bitcast(i16)[:, :, :, :, 0::2], func=AF.Sin,
                    bias=0.0, scale=sin_scale,
                )
                tk = trig.tile([128, HB, 2, NB, TK], f16, tag="tk", name="tk")
                nc.scalar.activation(
                    out=tk, in_=nk.bitcast(i16)[:, :, :, :, 0::2], func=AF.Sin,
                    bias=0.0, scale=sin_scale,
                )
                # zero-contribution keep-warm matmuls, one per fresh trig
                # tile: they space out through the mainloop and stop the PE
                # p-state ramp from resetting during its long idle, so the
                # final matmul burst runs at full clock. Skipped for the
                # last term so they don't steal the burst's first slot.
                if m < M - 1:
                    for rhs in (tq[:, 0, 0, 0, 0:1], tk[:, 0, 0, 0, 0:1]):
                        nc.tensor.matmul(
                            sT[0][0:1, 0, 0:1], lhsT=zeros16, rhs=rhs,
                            start=False, stop=False, skip_group_check=True,
                        )
                return tq, tk

            def emit_reduce(m, tq, tk):
                """Fold alpha_m*wv into the q side, then accumulate the
                transposed scores: sT[b][kc] += tkc.T @ (fold*sin q)
                + tks.T @ (fold*cos q)."""
                AC = trig.tile([128, HB, 2, NB, TQ], f16, tag="AC", name="AC")
                for hb in range(HB):
                    nc.gpsimd.tensor_scalar(
                        out=AC[:, hb], in0=tq[:, hb],
                        scalar1=fold_sb[:, hb, m : m + 1], scalar2=None, op0=ALU.mult,
                    )
                for b in range(NB):
                    for kc in range(2):
                        for hb in range(HB):
                            ks = slice(kc * 128, (kc + 1) * 128)
                            nc.tensor.matmul(
                                sT[b][:, kc, :],
                                lhsT=tk[:, hb, 1, b, ks],
                                rhs=AC[:, hb, 0, b, :],
                                start=(mm_i[b] == 0),
                                stop=(mm_i[b] == n_mm - 1),
                            )
                            mm_i[b] += 1
                            nc.tensor.matmul(
                                sT[b][:, kc, :],
                                lhsT=tk[:, hb, 0, b, ks],
                                rhs=AC[:, hb, 1, b, :],
                                start=(mm_i[b] == 0),
                                stop=(mm_i[b] == n_mm - 1),
                            )
                            mm_i[b] += 1

            # software-pipeline: folds+matmuls for term m are emitted after
            # phase m+1, so gpsimd's AC(m) doesn't stall the DVE->ACT chain.
            # The polynomial pieces slot in behind the m=0 emissions.
            pending = None
            for m in range(M):
                cur = emit_phase(m)
                if mixed and m == 0:
                    emit_poly_powers()
                if pending is not None:
                    emit_reduce(*pending)
                    if mixed and pending[0] == 0:
                        emit_poly_folds()
                        emit_poly_matmuls()
                pending = (m, *cur)
            emit_reduce(*pending)

            # ---- masked softmax + attn @ values, per batch ----
            # The mask lives entirely in the Exp activation's per-partition
            # scale/bias (k is the partition dim of sT): masked rows get
            # bias=-1e6 (exp -> exactly 0), valid_len==0 batches get
            # scale=0,bias=0 (uniform attention). exp_shift is folded into
            # bias host-side. Row sums over k via a ones-matmul; the 1/sum
            # is applied per-partition (q) on the PSUM->SBUF output copy.
            out_sb = soft.tile([128, NB, DV], f32, tag="out_sb", name="out_sb")
            e16 = soft.tile([128, NB, 2, TQ], f16, tag="e16", name="e16")
            for b in range(NB):
                nc.scalar.activation(
                    out=e16[:, b], in_=sT[b], func=AF.Exp,
                    bias=bias_exp, scale=1.0,
                )
            # per-batch accumulator tile: value columns + a sums column in
            # one psum bank, so each batch's group stops at its OWN last
            # matmul and its reciprocal/normalize isn't gated on the other
            # batch. Order per batch: value kc0 (start) ... sums kc1 (stop).
            tails = [
                ptail.tile([128, DV + 1], f32, tag=f"tail{b}", name=f"tail{b}")
                for b in range(NB)
            ]
            for b in range(NB):
                for kc in range(2):
                    nc.tensor.matmul(
                        tails[b][:, 0:DV],
                        lhsT=e16[:, b, kc, :],
                        rhs=pv[:, b, kc, :],
                        start=(kc == 0),
                        stop=False,
                    )
                    nc.tensor.matmul(
                        tails[b][:, DV : DV + 1],
                        lhsT=e16[:, b, kc, :],
                        rhs=ones16,
                        start=False,
                        stop=(kc == 1),
                    )
            # b1's normalize runs on the scalar engine (Identity with a
            # per-partition scale) so it isn't queued behind b0's on DVE;
            # separate out tiles avoid a cross-engine whole-tile WAW stall.
            rr = soft.tile([128, NB], f32, tag="r", name="r")
            nc.vector.reciprocal(out=rr[:, 0:1], in_=tails[0][:, DV : DV + 1])
            nc.vector.tensor_scalar(
                out=out_sb[:, 0, :], in0=tails[0][:, 0:DV], scalar1=rr[:, 0:1],
                scalar2=None, op0=ALU.mult,
            )
            nc.sync.dma_start(out=out_d[0], in_=out_sb[:, 0, :])
            out_sb1 = soft.tile([128, DV], f32, tag="out_sb1", name="out_sb1")
            nc.vector.reciprocal(out=rr[:, 1:2], in_=tails[1][:, DV : DV + 1])
            nc.vector.tensor_scalar(
                out=out_sb1, in0=tails[1][:, 0:DV], scalar1=rr[:, 1:2],
                scalar2=None, op0=ALU.mult,
            )
            nc.sync.dma_start(out=out_d[1], in_=out_sb1)

    nc.compile()
    return nc


def prepare_in_maps(queries, keys, values, valid_lens, wq, wk, wv, alpha,
                    exp_shift, c1=0.0, c3=0.0):
    """Host-side sharding + layout transforms. Returns list of 8 input dicts."""
    M = len(alpha)
    mixed = c3 != 0.0
    queries = np.ascontiguousarray(queries, dtype=np.float32)
    keys = np.ascontiguousarray(keys, dtype=np.float32)
    values = np.ascontiguousarray(values, dtype=np.float32)
    wq16 = np.ascontiguousarray(wq, dtype=np.float16)
    wk16 = np.ascontiguousarray(wk, dtype=np.float16)
    wv = np.asarray(wv, dtype=np.float32)
    valid_lens = np.asarray(valid_lens)

    # fold[p, hb, m] = alpha_m * wv[hb*128 + p]
    fold = np.empty((128, HB, M), np.float32)
    for hb in range(HB):
        fold[:, hb, :] = np.asarray(alpha, np.float64)[None, :] * wv[
            hb * 128 : (hb + 1) * 128, None
        ]

    CNEG = -60000.0  # f16-representable; exp underflows to exactly 0
    karange = np.arange(128)
    in_maps = []
    for c in range(NCORES):
        bs = slice(c * NB, (c + 1) * NB)
        qT = queries[bs].transpose(2, 0, 1).reshape(DIN, NB, TQ).copy()
        kT = keys[bs].transpose(2, 0, 1).reshape(DIN, NB, TK).copy()
        # additive mask rows per (b, kchunk); valid_len==0 batches get
        # zeroed q/k (scores==0 exactly) + zero mask -> uniform attention
        cmask = np.zeros((NB, 2, 128), np.float16)
        for j, vl in enumerate(valid_lens[bs]):
            vl = int(vl)
            if vl <= 0:
                qT[:, j] = 0.0
                kT[:, j] = 0.0
            else:
                for kc in range(2):
                    cmask[j, kc] = np.where(
                        (kc * 128 + karange) < vl, 0.0, CNEG
                    ).astype(np.float16)
        p64 = [wq16, qT.reshape(DIN, NB * TQ).astype(np.float16), wk16,
               kT.reshape(DIN, NB * TK).astype(np.float16)]
        pcc = [fold.reshape(128, HB * M)]
        if mixed:
            f3a = np.empty((128, HB), np.float32)
            for hb in range(HB):
                f3a[:, hb] = 3.0 * c3 * wv[hb * 128 : (hb + 1) * 128]
            pcc += [f3a]
            # pure-k part of the polynomial, precontracted with wv on the
            # host: vk[k] = sum_h wv_h * (c1*kp + c3*kp^3); folded into the
            # additive cmask rows (zeroed-key batches get vk == 0)
            kp = kT.reshape(DIN, NB * TK).T.astype(np.float64) @ wk.astype(
                np.float64
            )
            vk = (c1 * kp + c3 * kp**3) @ wv.astype(np.float64)
            cmask += vk.reshape(NB, 2, 128).astype(np.float16)
        pack64a = np.concatenate(p64[:2], axis=1)
        pack64b = np.concatenate(p64[2:], axis=1)
        vals = values[bs].reshape(NB, 2, 128, DV).transpose(2, 0, 1, 3)
        packv = vals.reshape(128, NB * 2 * DV).astype(np.float16)
        packc = np.concatenate(pcc, axis=1).astype(np.float32)
        in_maps.append(
            {
                "pack64a": np.ascontiguousarray(pack64a),
                "pack64b": np.ascontiguousarray(pack64b),
                "packc": np.ascontiguousarray(packc),
                "cmask": np.ascontiguousarray(cmask.reshape(1, NB * 2 * 128)),
                "packv": np.ascontiguousarray(packv),
            }
        )
    return in_maps


def _pick_fit(queries, keys, wq, wk):
    q = queries.reshape(-1, DIN).astype(np.float32) @ wq.astype(np.float32)
    k = keys.reshape(-1, DIN).astype(np.float32) @ wk.astype(np.float32)
    qb = q.reshape(B, TQ, H)
    kb = k.reshape(B, TK, H)
    hi = (qb.max(1) + kb.max(1)).max()
    lo = (qb.min(1) + kb.min(1)).min()
    r_needed = max(abs(hi), abs(lo))
    for ent in FITS:
        if ent[0] >= r_needed + 0.05:
            break
    else:
        ent = FITS[-1]
    R, alpha, omega = ent[0], ent[1], ent[2]
    c1 = ent[3] if len(ent) > 3 else 0.0
    c3 = ent[4] if len(ent) > 4 else 0.0
    return R, alpha, omega, c1, c3


_prog_cache = {}


def kernel(queries, keys, values, valid_lens, wq, wk, wv):
    from concourse import bass_utils

    queries = np.asarray(queries)
    keys = np.asarray(keys)
    values = np.asarray(values)
    valid_lens = np.asarray(valid_lens)
    wq = np.asarray(wq)
    wk = np.asarray(wk)
    wv = np.asarray(wv)

    R, alpha, omega, c1, c3 = _pick_fit(queries, keys, wq, wk)
    # scores bounded by ~sum|wv| * max|approx tanh|; keep exp(score) within
    # fp16 range (e is stored as fp16)
    bound = float(np.abs(wv).sum()) * 1.01
    exp_shift = max(0.0, bound - 10.0)

    key = (R, len(alpha), round(exp_shift, 3))
    if key not in _prog_cache:
        _prog_cache[key] = build_program(alpha, omega, exp_shift, c1, c3)
    nc = _prog_cache[key]

    in_maps = prepare_in_maps(
        queries, keys, values, valid_lens, wq, wk, wv, alpha, exp_shift, c1, c3
    )
    res = bass_utils.run_bass_kernel_spmd(nc, in_maps, core_ids=list(range(NCORES)))
    out = np.concatenate([r["out"] for r in res.results], axis=0)
    return out.astype(np.float32)


if __name__ == "__main__":
    rng = np.random.default_rng(0)
    inputs = {
        "queries": rng.standard_normal((B, TQ, DIN), dtype=np.float32),
        "keys": rng.standard_normal((B, TK, DIN), dtype=np.float32),
        "values": rng.standard_normal((B, TK, DV), dtype=np.float32),
        "valid_lens": rng.integers(0, TK, size=(B,)).astype(np.int32),
        "wq": (rng.standard_normal((DIN, H), dtype=np.float32) * 0.05),
        "wk": (rng.standard_normal((DIN, H), dtype=np.float32) * 0.05),
        "wv": (rng.standard_normal((H,), dtype=np.float32) * 0.05),
    }
    out = kernel(**inputs)
    print("out", out.shape, out.dtype)


# revision 81
# speedup vs baseline: 1.0332x; 1.0073x over previous
"""Bahdanau attention Trainium2 kernel.

Math: out = softmax_k(mask(score)) @ values with
  score[b,q,k] = sum_h wv[h] * tanh(Q[b,q,h] + K[b,k,h]),
  Q = queries @ wq, K = keys @ wk.

tanh(x) is approximated by a mixed polynomial + free-frequency sine
basis
  tanh(x) ~= c1*x + c3*x^3 + sum_m alpha_m sin(omega_m x)
(coefficients and omegas jointly optimized per fit range offline,
hardcoded below). Each sine term factorizes through
sin(omega(q+k)) = sin(wq)cos(wk) + cos(wq)sin(wk) into dense
[Tq,H]x[H,Tk] matmuls on the PE array; the polynomial part expands
binomially: pure-q terms are constant along the softmax axis and
cancel, the pure-k rank-1 part (c1*k + c3*k^3 contracted with wv) is
precomputed on the host and folded into the additive-mask rows, and
only the cross terms 3c3*(q^2 k + q k^2) run on device as fp16 PE
matmul channels. This leaves just TWO Sin evaluations on the
activation engine, which is the critical resource.

The scalar engine's Sin is only valid on [-pi, pi], so arguments are
range-reduced in 16-bit fixed point using the HW's round-to-nearest
fp32->int32 conversion:
  n = round(x * omega/(2pi) * 65536)     (one tensor_scalar, int32 out)
ACT then reads the LOW int16 half of each int32 via a bitcast +
stride-2 AP; the SIGNED int16 view puts the phase in [-pi, pi)
directly (sin arg = v16 * 2pi/65536, no bias).

Scores are accumulated TRANSPOSED (scoresT[k, q], k on partitions, two
128-row chunks per batch) by swapping matmul lhsT/rhs. This removes the
tail transpose entirely and lets the masked softmax fold completely into
the Exp activation: exp(score*scale + bias) with per-partition
scale/bias tiles carrying the valid_len mask (masked k rows get
bias=-1e6 -> exp underflows to 0; rows with valid_len==0 get scale=0,
bias=0 -> uniform attention, matching the reference). e is written in
fp16 so the attn@values and row-sum matmuls run at full PE rate; the
1/sum normalization is applied per-partition on the final PSUM->SBUF
copy.

All PE inputs (wq/wk, qT/kT, trig factors, e, values) are fp16:
1 cycle/row on the PE vs 4 for fp32, and half the DMA bytes.

Sharding: data-parallel over batch, 2 batches per core on 8 cores.
"""

import math
import sys

import numpy as np

sys.path.insert(0, "/opt/trn_rl_repo")

B, TQ, TK, DIN, H, DV = 16, 128, 256, 64, 256, 256
NCORES = 8
NB = B // NCORES
HB = 2  # h blocks of 128 partitions
NEG = -1000000.0
PI = math.pi
FX = 65536  # fixed-point phase resolution

# (fit range R, [alpha_m], [omega_m]) — offline weighted least squares with
# jointly optimized frequencies (Nelder-Mead over log omega).
FITS = [
    # R=4.25 mixed basis {x, x^3, sin w1 x, sin w2 x}: the polynomial part
    # factorizes into cheap PE channels (binomial expansion; pure-q terms
    # cancel in the softmax), leaving only TWO Sin evaluations on ACT.
    (4.25, [0.02924106, 0.20011121], [2.98939508, 1.61959803],
     0.57994674, -0.02086779),
    # R=4.25 M=3, weights tuned for end-to-end error on randn inputs
    (4.25, [1.14215814, 0.16947486, 0.02756034], [0.54450034, 1.69464501, 3.01720302]),
    # R=4.35 M=4 fit_max=2.65e-03
    (4.35, [1.15424910, 0.18579149, 0.03183397, 0.00486786], [0.51479938, 1.59132282, 2.77293209, 4.09736273]),
    # R=5.0 M=5 fit_max=1.15e-03
    (5.0, [1.17638592, 0.21722942, 0.04686229, 0.00903701, 0.00168118], [0.45910491, 1.40743013, 2.42317614, 3.51883170, 4.71208021]),
    # R=5.75 M=5 fit_max=2.69e-03
    (5.75, [1.19166771, 0.24153365, 0.06066643, 0.01419093, 0.00340621], [0.41816285, 1.27578909, 2.18259325, 3.14991184, 4.18200973]),
    # R=6.5 M=6 fit_max=2.18e-03
    (6.5, [1.22246911, 0.29583495, 0.09660808, 0.02984660, 0.00854484, 0.00257873], [0.33487488, 1.02337718, 1.75741117, 2.54559399, 3.38983334, 4.27904363]),
    # R=7.5 M=7 fit_max=2.83e-03
    (7.5, [1.23017784, 0.32207557, 0.13281313, 0.05605996, 0.02115424, 0.00734471, 0.00271091], [0.27799558, 0.82769422, 1.38567976, 1.99585889, 2.67159272, 3.40357483, 4.16577676]),
    # R=9.0 M=8 fit_max=3.42e-03
    (9.0, [1.23220768, 0.32660174, -0.09537761, 0.19814386, 0.04758848, 0.01766320, 0.00668698, 0.00282454], [0.29504675, 0.89849354, 1.31818698, 1.42932216, 2.09287727, 2.73674004, 3.39807058, 4.05802853]),
    # R=11.0 M=10 fit_max=2.34e-03
    (11.0, [1.25009981, 0.36156082, 0.16668645, 0.08288622, 0.04121021, 0.02000684, 0.00953954, 0.00468461, 0.00244755, -0.00000540], [0.21481795, 0.64601870, 1.08287839, 1.53102994, 1.99552922, 2.47865898, 2.98018571, 3.49646094, 4.00027643, 27.89010198]),
    # R=13.5 M=12 fit_max=1.65e-03
    (13.5, [1.25071458, 0.35628586, 0.11460491, 0.10187407, 0.14817974, -0.36003900, 0.32427418, 0.02462988, 0.01227283, 0.00633834, 0.00344839, 0.00201855], [0.19982948, 0.59368416, 0.91929799, 1.13521056, 1.56105869, 1.71511767, 1.75607839, 2.27997771, 2.72042538, 3.16077613, 3.60347782, 4.03539051]),
    # R=16.5 M=14 fit_max=2.77e-03
    (16.5, [1.25849188, 0.38272733, 0.19240301, 0.10549262, 0.02066639, 0.04720551, 0.04224601, -0.03774592, 0.04760619, 0.01225844, 0.00687190, 0.00399597, 0.01283338, -0.03588798, 0.02641497, 0.00090983, 0.00029855], [0.17064909, 0.51252204, 0.85576113, 1.19857872, 1.46425403, 1.59143819, 1.94973110, 2.13344383, 2.20286683, 2.61053434, 2.98029452, 3.35535440, 3.82535729, 3.89323910, 3.92054336, 4.37490434, 6.06447818]),
    # R=20.0 M=24 fit_max=8.08e-03
    (20.0, [1.26310001, 0.39699981, 0.50851750, -0.19663028, -0.15967006, 0.16325805, 0.07906020, 0.04867188, 0.03041830, 0.01901565, 0.01861664, -0.00714429, 0.00795889, 0.00512261, 0.00317850, 0.00126551, 0.00070946, 0.00065840, 0.00017538, -0.00000124, -0.00006095, -0.00011474, 0.00012987, -0.00002543], [0.14670123, 0.44208486, 0.76888929, 0.77629792, 0.84403806, 0.98925559, 1.30465383, 1.60553746, 1.90541734, 2.20846370, 2.53020022, 2.56316326, 2.81407758, 3.11334895, 3.41541978, 4.41970952, 4.63745933, 5.05356013, 6.39364912, 6.87284072, 7.81029508, 10.65968755, 10.76481947, 18.82746599]),
]

PQK = 2 * H + NB * TQ + NB * TK  # wq | qT | wk | kT   (fp16, 64 partitions)


def build_program(alpha, omega, exp_shift=0.0, c1=0.0, c3=0.0):
    """Build the per-core Bass program."""
    import concourse.bacc as bacc
    import concourse.bass as bass
    import concourse.mybir as mybir
    import concourse.tile as tile

    f32 = mybir.dt.float32
    f16 = mybir.dt.float16
    i32 = mybir.dt.int32
    i16 = mybir.dt.int16
    AF = mybir.ActivationFunctionType
    ALU = mybir.AluOpType

    M = len(alpha)
    mixed = c3 != 0.0
    CC = HB * M + (HB if mixed else 0)  # sine folds | 3c3*wv
    PQKX = PQK

    nc = bacc.Bacc("TRN2", target_bir_lowering=False, debug=False)

    QHALF = H + NB * TQ  # wq | qT
    pack64a_d = nc.dram_tensor("pack64a", [DIN, QHALF], f16, kind="ExternalInput").ap()
    pack64b_d = nc.dram_tensor("pack64b", [DIN, PQKX - QHALF], f16, kind="ExternalInput").ap()
    packc_d = nc.dram_tensor("packc", [128, CC], f32, kind="ExternalInput").ap()
    cmask_d = nc.dram_tensor("cmask", [1, NB * 2 * 128], f16, kind="ExternalInput").ap()
    packv_d = nc.dram_tensor("packv", [128, NB * 2 * DV], f16, kind="ExternalInput").ap()
    out_d = nc.dram_tensor("out", [NB, TQ, DV], f32, kind="ExternalOutput").ap()

    with tile.TileContext(nc) as tc:
        with (
            tc.tile_pool(name="singles", bufs=1) as singles,
            tc.tile_pool(name="trig", bufs=3) as trig,
            tc.tile_pool(name="soft", bufs=2) as soft,
            tc.tile_pool(name="pproj", bufs=1, space="PSUM") as pproj,
            tc.tile_pool(name="pscore", bufs=1, space="PSUM") as pscore,
            tc.tile_pool(name="ptail", bufs=1, space="PSUM") as ptail,
        ):
            # ---- constants / inputs to SBUF ----
            warm_t = singles.tile([128, 1], f32)
            nc.vector.memset(warm_t, 0.0)
            ones16 = singles.tile([128, 1], f16)
            nc.vector.memset(ones16, 1.0)
            zeros16 = singles.tile([128, 1], f16)
            nc.vector.memset(zeros16, 0.0)
            onesrow = singles.tile([1, TQ], f16)
            nc.vector.memset(onesrow, 1.0)
            bias_exp = singles.tile([128, 1], f32)
            nc.vector.memset(bias_exp, -float(exp_shift))

            pk64 = singles.tile([DIN, PQKX], f16)
            # q-half on the SP HWDGE queue; k-half via the Pool engine's
            # SWDGE path so the two triggers don't serialize on the
            # single-slot HWDGE and the k projection isn't DMA-gated
            nc.sync.dma_start(out=pk64[:, 0:QHALF], in_=pack64a_d)
            nc.gpsimd.dma_start(out=pk64[:, QHALF:], in_=pack64b_d)
            pc = singles.tile([128, CC], f32)
            nc.sync.dma_start(out=pc, in_=packc_d)
            cmask = singles.tile([1, NB * 2 * 128], f16)
            nc.sync.dma_start(out=cmask, in_=cmask_d)
            pv = singles.tile([128, NB, 2, DV], f16)
            nc.sync.dma_start(
                out=pv,
                in_=bass.AP(tensor=packv_d.tensor, offset=0, ap=[[NB * 2 * DV, 128], [1, NB * 2 * DV]]),
            )

            wq_sb = pk64[:, 0:H]
            qTs = pk64[:, H : H + NB * TQ].rearrange("p (b x) -> p b x", b=NB)
            wk_sb = pk64[:, H + NB * TQ : 2 * H + NB * TQ]
            kTs = pk64[:, 2 * H + NB * TQ : PQK].rearrange(
                "p (b x) -> p b x", b=NB
            )
            fold_sb = pc[:, 0 : HB * M].rearrange("p (hb m) -> p hb m", hb=HB)

            # dummy Sin before the scalar-engine copies: pins the first
            # (startup-hidden) act-table load to the trig table so the Copy
            # activations below don't cause an extra mid-program load
            dummy_sin = singles.tile([128, 1], f16)
            nc.scalar.activation(
                out=dummy_sin, in_=warm_t, func=AF.Sin, bias=0.0, scale=1.0
            )

            # ---- projections (PE, fp16 in / fp32 psum): [h, hb, b, qi/ki] ----
            # tiny warm-up matmul first (into the qT region, re-zeroed by the
            # real projection's start): begins the PE p-state ramp early
            qT_ps = pproj.tile([128, HB, NB, TQ], f32)
            nc.tensor.matmul(
                qT_ps[0:1, 0, 0, 0:1], lhsT=warm_t, rhs=warm_t,
                start=True, stop=True, skip_group_check=True,
            )
            for hb in range(HB):
                nc.tensor.matmul(
                    qT_ps[:, hb, :, :].rearrange("p b x -> p (b x)"),
                    lhsT=wq_sb[:, hb * 128 : (hb + 1) * 128],
                    rhs=qTs.rearrange("p b x -> p (b x)"),
                    start=(hb == 0),
                    stop=(hb == HB - 1),
                )
            # kT_ps spans two 2KB psum zero regions (one per hb slice); one
            # 512-col matmul per region covers both batches (the kT columns
            # are contiguous in pk64), halving the dispatch count on the
            # chain that feeds the scalar-engine kTp copy.
            kT_ps = pproj.tile([128, HB, NB, TK], f32)
            for hb in range(HB):
                nc.tensor.matmul(
                    kT_ps[:, hb].rearrange("p b x -> p (b x)"),
                    lhsT=wk_sb[:, hb * 128 : (hb + 1) * 128],
                    rhs=pk64[:, 2 * H + NB * TQ : PQK],
                    start=True,
                    stop=True,
                )

            # both PSUM->SBUF copies run on the (otherwise idle) scalar
            # engine so the DVE goes straight to the m=0 phase converts
            qTp = singles.tile([128, HB, NB, TQ], f32)
            nc.scalar.copy(out=qTp, in_=qT_ps)
            kTp = singles.tile([128, HB, NB, TK], f32)
            nc.scalar.copy(out=kTp, in_=kT_ps)

            # ---- transposed score accumulation over m sine terms ----
            # sT[b][k', kc, q] : k = kc*128 + k' on partitions. One 2KB psum
            # bank (= one zero region) per batch; the kc slices interleave
            # inside a single accumulation group bracketed by the batch's
            # first (start) and last (stop) matmul, like the baseline's
            # kT_ps hb regions.
            sT = [
                pscore.tile([128, 2, TQ], f32, tag=f"sT{b}", name=f"sT{b}")
                for b in range(NB)
            ]
            # per batch: 2 mask adds [+ 8 cubic channels] + sines
            n_mm = M * HB * 2 * 2 + 2 + (8 if mixed else 0)
            mm_i = [0] * NB
            sin_scale = 2.0 * PI / FX

            # additive softmax mask seeded into the score accumulators by
            # tiny 1-partition matmuls (c[k] x ones[q]) while the PE is
            # otherwise idle; runs right after the input DMA lands
            for b in range(NB):
                for kc in range(2):
                    r0 = (b * 2 + kc) * 128
                    nc.tensor.matmul(
                        sT[b][:, kc, :],
                        lhsT=cmask[0:1, r0 : r0 + 128],
                        rhs=onesrow,
                        start=(mm_i[b] == 0),
                        stop=False,
                    )
                    mm_i[b] += 1

            # ---- polynomial part of the mixed tanh basis ----
            # c1*x + c3*x^3 with x = q+k expands binomially: pure-q terms
            # cancel in the softmax, the pure-k rank-1 (c1*k + c3*k^3 through
            # wv) is folded into the cmask rows on the HOST, and only the
            # cross channels 3c3*(q^2 k + q k^2) run here as f16 PE matmuls.
            if mixed:
                f3a = pc[:, HB * M : HB * M + HB]  # 3*c3*wv per hb
                q2 = singles.tile([128, HB, NB, TQ], f32)
                k2 = singles.tile([128, HB, NB, TK], f32)
                kp16 = singles.tile([128, HB, NB, TK], f16)
                k2_16 = singles.tile([128, HB, NB, TK], f16)
                fq2 = singles.tile([128, HB, NB, TQ], f16)
                fq1 = singles.tile([128, HB, NB, TQ], f16)

            def emit_poly_powers():
                # q square on gpsimd (ready before the m0 AC folds need
                # Pool); k square on the DVE after the m0 converts
                nc.gpsimd.tensor_tensor(out=q2, in0=qTp, in1=qTp, op=ALU.mult)

            def emit_poly_folds():
                for hb in range(HB):
                    nc.gpsimd.tensor_scalar(
                        out=fq2[:, hb], in0=q2[:, hb],
                        scalar1=f3a[:, hb : hb + 1], scalar2=None, op0=ALU.mult,
                    )
                    nc.gpsimd.tensor_scalar(
                        out=fq1[:, hb], in0=qTp[:, hb],
                        scalar1=f3a[:, hb : hb + 1], scalar2=None, op0=ALU.mult,
                    )
                # k^2 squared directly in f16 (2x DVE mode, and it skips the
                # slow 1x f32 TensorTensor) so the poly channel matmuls are
                # ready BEFORE the last harmonic's burst window
                nc.vector.tensor_copy(out=kp16, in_=kTp)
                nc.vector.tensor_tensor(out=k2_16, in0=kp16, in1=kp16, op=ALU.mult)

            def emit_poly_matmuls():
                for b in range(NB):
                    for kc in range(2):
                        ks = slice(kc * 128, (kc + 1) * 128)
                        for hb in range(HB):
                            nc.tensor.matmul(
                                sT[b][:, kc, :],
                                lhsT=kp16[:, hb, b, ks],
                                rhs=fq2[:, hb, b, :],
                                start=False, stop=False,
                            )
                            mm_i[b] += 1
                            nc.tensor.matmul(
                                sT[b][:, kc, :],
                                lhsT=k2_16[:, hb, b, ks],
                                rhs=fq1[:, hb, b, :],
                                start=False, stop=False,
                            )
                            mm_i[b] += 1

            def emit_phase(m):
                """DVE int phase converts + ACT Sin for sine term m.

                Phase tiles hold [hb, sin|cos, b, x] so one ACT Sin per side
                sweeps both quadratures. ACT reads the LOW int16 half of each
                int32 via a bitcast + stride-2 AP; the SIGNED int16 view puts
                the phase in [-pi, pi) directly."""
                w16 = float(np.float32(omega[m] / (2.0 * PI) * FX))
                nq = trig.tile([128, HB, 2, NB, TQ], i32, tag="nq", name="nq")
                nc.vector.tensor_scalar(
                    out=nq[:, :, 0], in0=qTp, scalar1=w16, scalar2=None, op0=ALU.mult
                )
                nc.vector.tensor_scalar(
                    out=nq[:, :, 1], in0=qTp, scalar1=w16, scalar2=float(FX // 4),
                    op0=ALU.mult, op1=ALU.add,
                )
                nk = trig.tile([128, HB, 2, NB, TK], i32, tag="nk", name="nk")
                nc.vector.tensor_scalar(
                    out=nk[:, :, 0], in0=kTp, scalar1=w16, scalar2=None, op0=ALU.mult
                )
                nc.vector.tensor_scalar(
                    out=nk[:, :, 1], in0=kTp, scalar1=w16, scalar2=float(FX // 4),
                    op0=ALU.mult, op1=ALU.add,
                )
                # t[:,hb,0] = sin(w x), t[:,hb,1] = cos(w x)
                tq = trig.tile([128, HB, 2, NB, TQ], f16, tag="tq", name="tq")
                nc.scalar.activation(
                    out=tq, in_=nq.bitcast(i16)[:, :, :, :, 0::2], func=AF.Sin,
                    bias=0.0, scale=sin_scale,
                )
                tk = trig.tile([128, HB, 2, NB, TK], f16, tag="tk", name="tk")
                nc.scalar.activation(
                    out=tk, in_=nk.bitcast(i16)[:, :, :, :, 0::2], func=AF.Sin,
                    bias=0.0, scale=sin_scale,
                )
                # zero-contribution keep-warm matmuls, one per fresh trig
                # tile: they space out through the mainloop and stop the PE
                # p-state ramp from resetting during its long idle, so the
                # final matmul burst runs at full clock. Skipped for the
                # last term so they don't steal the burst's first slot.
                if m < M - 1:
                    for rhs in (tq[:, 0, 0, 0, 0:1], tk[:, 0, 0, 0, 0:1]):
                        nc.tensor.matmul(
                            sT[0][0:1, 0, 0:1], lhsT=zeros16, rhs=rhs,
                            start=False, stop=False, skip_group_check=True,
                        )
                return tq, tk

            def emit_reduce(m, tq, tk):
                """Fold alpha_m*wv into the q side, then accumulate the
                transposed scores: sT[b][kc] += tkc.T @ (fold*sin q)
                + tks.T @ (fold*cos q)."""
                AC = trig.tile([128, HB, 2, NB, TQ], f16, tag="AC", name="AC")
                for hb in range(HB):
                    nc.gpsimd.tensor_scalar(
                        out=AC[:, hb], in0=tq[:, hb],
                        scalar1=fold_sb[:, hb, m : m + 1], scalar2=None, op0=ALU.mult,
                    )
                for b in range(NB):
                    for kc in range(2):
                        for hb in range(HB):
                            ks = slice(kc * 128, (kc + 1) * 128)
                            nc.tensor.matmul(
                                sT[b][:, kc, :],
                                lhsT=tk[:, hb, 1, b, ks],
                                rhs=AC[:, hb, 0, b, :],
                                start=(mm_i[b] == 0),
                                stop=(mm_i[b] == n_mm - 1),
                            )
                            mm_i[b] += 1
                            nc.tensor.matmul(
                                sT[b][:, kc, :],
                                lhsT=tk[:, hb, 0, b, ks],
                                rhs=AC[:, hb, 1, b, :],
                                start=(mm_i[b] == 0),
                                stop=(mm_i[b] == n_mm - 1),
                            )
                            mm_i[b] += 1

            # software-pipeline: folds+matmuls for term m are emitted after
            # phase m+1, so gpsimd's AC(m) doesn't stall the DVE->ACT chain.
            # The polynomial pieces slot in behind the m=0 emissions.
            pending = None
            for m in range(M):
                cur = emit_phase(m)
                if mixed and m == 0:
                    emit_poly_powers()
                if pending is not None:
                    emit_reduce(*pending)
                    if mixed and pending[0] == 0:
                        emit_poly_folds()
                        emit_poly_matmuls()
                pending = (m, *cur)
            emit_reduce(*pending)

            # ---- masked softmax + attn @ values, per batch ----
            # The mask lives entirely in the Exp activation's per-partition
            # scale/bias (k is the partition dim of sT): masked rows get
            # bias=-1e6 (exp -> exactly 0), valid_len==0 batches get
            # scale=0,bias=0 (uniform attention). exp_shift is folded into
            # bias host-side. Row sums over k via a ones-matmul; the 1/sum
            # is applied per-partition (q) on the PSUM->SBUF output copy.
            out_sb = soft.tile([128, NB, DV], f32, tag="out_sb", name="out_sb")
            e16 = soft.tile([128, NB, 2, TQ], f16, tag="e16", name="e16")
            for b in range(NB):
                nc.scalar.activation(
                    out=e16[:, b], in_=sT[b], func=AF.Exp,
                    bias=bias_exp, scale=1.0,
                )
            # per-batch accumulator tile: value columns + a sums column in
            # one psum bank, so each batch's group stops at its OWN last
            # matmul and its reciprocal/normalize isn't gated on the other
            # batch. Order per batch: value kc0 (start) ... sums kc1 (stop).
            tails = [
                ptail.tile([128, DV + 1], f32, tag=f"tail{b}", name=f"tail{b}")
                for b in range(NB)
            ]
            for b in range(NB):
                for kc in range(2):
                    nc.tensor.matmul(
                        tails[b][:, 0:DV],
                        lhsT=e16[:, b, kc, :],
                        rhs=pv[:, b, kc, :],
                        start=(kc == 0),
                        stop=False,
                    )
                    nc.tensor.matmul(
                        tails[b][:, DV : DV + 1],
                        lhsT=e16[:, b, kc, :],
                        rhs=ones16,
                        start=False,
                        stop=(kc == 1),
                    )
            # b1's normalize runs on the scalar engine (Identity with a
            # per-partition scale) so it isn't queued behind b0's on DVE;
            # separate out tiles avoid a cross-engine whole-tile WAW stall.
            rr = soft.tile([128, NB], f32, tag="r", name="r")
            nc.vector.reciprocal(out=rr[:, 0:1], in_=tails[0][:, DV : DV + 1])
            nc.vector.tensor_scalar(
                out=out_sb[:, 0, :], in0=tails[0][:, 0:DV], scalar1=rr[:, 0:1],
                scalar2=None, op0=ALU.mult,
            )
            nc.sync.dma_start(out=out_d[0], in_=out_sb[:, 0, :])
            out_sb1 = soft.tile([128, DV], f32, tag="out_sb1", name="out_sb1")
            nc.vector.reciprocal(out=rr[:, 1:2], in_=tails[1][:, DV : DV + 1])
            nc.vector.tensor_scalar(
                out=out_sb1, in0=tails[1][:, 0:DV], scalar1=rr[:, 1:2],
                scalar2=None, op0=ALU.mult,
            )
            nc.sync.dma_start(out=out_d[1], in_=out_sb1)

    nc.compile()
    return nc


def prepare_in_maps(queries, keys, values, valid_lens, wq, wk, wv, alpha,
                    exp_shift, c1=0.0, c3=0.0):
    """Host-side sharding + layout transforms. Returns list of 8 input dicts."""
    M = len(alpha)
    mixed = c3 != 0.0
    queries = np.ascontiguousarray(queries, dtype=np.float32)
    keys = np.ascontiguousarray(keys, dtype=np.float32)
    values = np.ascontiguousarray(values, dtype=np.float32)
    wq16 = np.ascontiguousarray(wq, dtype=np.float16)
    wk16 = np.ascontiguousarray(wk, dtype=np.float16)
    wv = np.asarray(wv, dtype=np.float32)
    valid_lens = np.asarray(valid_lens)

    # fold[p, hb, m] = alpha_m * wv[hb*128 + p]
    fold = np.empty((128, HB, M), np.float32)
    for hb in range(HB):
        fold[:, hb, :] = np.asarray(alpha, np.float64)[None, :] * wv[
            hb * 128 : (hb + 1) * 128, None
        ]

    CNEG = -60000.0  # f16-representable; exp underflows to exactly 0
    karange = np.arange(128)
    in_maps = []
    for c in range(NCORES):
        bs = slice(c * NB, (c + 1) * NB)
        qT = queries[bs].transpose(2, 0, 1).reshape(DIN, NB, TQ).copy()
        kT = keys[bs].transpose(2, 0, 1).reshape(DIN, NB, TK).copy()
        # additive mask rows per (b, kchunk); valid_len==0 batches get
        # zeroed q/k (scores==0 exactly) + zero mask -> uniform attention
        cmask = np.zeros((NB, 2, 128), np.float16)
        for j, vl in enumerate(valid_lens[bs]):
            vl = int(vl)
            if vl <= 0:
                qT[:, j] = 0.0
                kT[:, j] = 0.0
            else:
                for kc in range(2):
                    cmask[j, kc] = np.where(
                        (kc * 128 + karange) < vl, 0.0, CNEG
                    ).astype(np.float16)
        p64 = [wq16, qT.reshape(DIN, NB * TQ).astype(np.float16), wk16,
               kT.reshape(DIN, NB * TK).astype(np.float16)]
        pcc = [fold.reshape(128, HB * M)]
        if mixed:
            f3a = np.empty((128, HB), np.float32)
            for hb in range(HB):
                f3a[:, hb] = 3.0 * c3 * wv[hb * 128 : (hb + 1) * 128]
            pcc += [f3a]
            # pure-k part of the polynomial, precontracted with wv on the
            # host: vk[k] = sum_h wv_h * (c1*kp + c3*kp^3); folded into the
            # additive cmask rows (zeroed-key batches get vk == 0)
            kp = kT.reshape(DIN, NB * TK).T.astype(np.float64) @ wk.astype(
                np.float64
            )
            vk = (c1 * kp + c3 * kp**3) @ wv.astype(np.float64)
            cmask += vk.reshape(NB, 2, 128).astype(np.float16)
        pack64a = np.concatenate(p64[:2], axis=1)
        pack64b = np.concatenate(p64[2:], axis=1)
        vals = values[bs].reshape(NB, 2, 128, DV).transpose(2, 0, 1, 3)
        packv = vals.reshape(128, NB * 2 * DV).astype(np.float16)
        packc = np.concatenate(pcc, axis=1).astype(np.float32)
        in_maps.append(
            {
                "pack64a": np.ascontiguousarray(pack64a),
                "pack64b": np.ascontiguousarray(pack64b),
                "packc": np.ascontiguousarray(packc),
                "cmask": np.ascontiguousarray(cmask.reshape(1, NB * 2 * 128)),
                "packv": np.ascontiguousarray(packv),
            }
        )
    return in_maps


def _pick_fit(queries, keys, wq, wk):
    q = queries.reshape(-1, DIN).astype(np.float32) @ wq.astype(np.float32)
    k = keys.reshape(-1, DIN).astype(np.float32) @ wk.astype(np.float32)
    qb = q.reshape(B, TQ, H)
    kb = k.reshape(B, TK, H)
    hi = (qb.max(1) + kb.max(1)).max()
    lo = (qb.min(1) + kb.min(1)).min()
    r_needed = max(abs(hi), abs(lo))
    for ent in FITS:
        if ent[0] >= r_needed + 0.05:
            break
    else:
        ent = FITS[-1]
    R, alpha, omega = ent[0], ent[1], ent[2]
    c1 = ent[3] if len(ent) > 3 else 0.0
    c3 = ent[4] if len(ent) > 4 else 0.0
    return R, alpha, omega, c1, c3


_prog_cache = {}


def kernel(queries, keys, values, valid_lens, wq, wk, wv):
    from concourse import bass_utils

    queries = np.asarray(queries)
    keys = np.asarray(keys)
    values = np.asarray(values)
    valid_lens = np.asarray(valid_lens)
    wq = np.asarray(wq)
    wk = np.asarray(wk)
    wv = np.asarray(wv)

    R, alpha, omega, c1, c3 = _pick_fit(queries, keys, wq, wk)
    # scores bounded by ~sum|wv| * max|approx tanh|; keep exp(score) within
    # fp16 range (e is stored as fp16)
    bound = float(np.abs(wv).sum()) * 1.01
    exp_shift = max(0.0, bound - 10.0)

    key = (R, len(alpha), round(exp_shift, 3))
    if key not in _prog_cache:
        _prog_cache[key] = build_program(alpha, omega, exp_shift, c1, c3)
    nc = _prog_cache[key]

    in_maps = prepare_in_maps(
        queries, keys, values, valid_lens, wq, wk, wv, alpha, exp_shift, c1, c3
    )
    res = bass_utils.run_bass_kernel_spmd(nc, in_maps, core_ids=list(range(NCORES)))
    out = np.concatenate([r["out"] for r in res.results], axis=0)
    return out.astype(np.float32)


if __name__ == "__main__":
    rng = np.random.default_rng(0)
    inputs = {
        "queries": rng.standard_normal((B, TQ, DIN), dtype=np.float32),
        "keys": rng.standard_normal((B, TK, DIN), dtype=np.float32),
        "values": rng.standard_normal((B, TK, DV), dtype=np.float32),
        "valid_lens": rng.integers(0, TK, size=(B,)).astype(np.int32),
        "wq": (rng.standard_normal((DIN, H), dtype=np.float32) * 0.05),
        "wk": (rng.standard_normal((DIN, H), dtype=np.float32) * 0.05),
        "wv": (rng.standard_normal((H,), dtype=np.float32) * 0.05),
    }
    out = kernel(**inputs)
    print("out", out.shape, out.dtype)
